# revision 1
# baseline (speedup 1.0000x reference)
"""Trainium2 Bass kernel for nn_DecoderLayer (Performer/FAVOR+ decoder layer).

Sharding: data-parallel over sequence (L) across 8 cores. The FAVOR+ attention
global statistics (kv = sum_l kp (x) v and ksum = sum_l kp, per batch element)
are the only cross-core quantities; they are AllReduced (~2MB) while the
query-side projections compute. Everything else (projections, LayerNorms, FFN)
is token-local.

Device layout: all activations live feature-major [D(partitions x chunks),
tokens(free)]; the host pre-transposes input shards and post-transposes the
output, so the device never transposes anything. Matmuls run as float32r
(full fp32 storage, ~1 cycle/row PE rate).
"""
import sys
import os

sys.path.insert(0, '/opt/trn_rl_repo')

import numpy as np
import ml_dtypes
from contextlib import ExitStack

from concourse import bass, bacc, tile
import concourse.mybir as mybir
from concourse.bass_utils import run_bass_kernel_spmd
from concourse.alu_op_type import AluOpType

F32 = mybir.dt.float32
F32R = mybir.dt.float32r
AF = mybir.ActivationFunctionType
BF16 = mybir.dt.bfloat16

B, L, D, H, DH, M, DFF = 4, 4096, 1024, 16, 64, 8, 4096
NCORES = 8
LSH = L // NCORES          # 512 tokens of L per core
T = B * LSH                # 2048 tokens per core
TB = LSH                   # token block = one batch element's shard (512)
DC = D // 128              # 8 d-chunks
HM = H * M                 # 128
EPS_LN = 1e-6
STAB = 0.001
NEWTON = True              # Newton-refine Rsqrt/Reciprocal LUT outputs

_cache = {}


def _mm(nc, out, lhsT, rhs, start, stop, skip_group_check=False):
    nc.tensor.matmul(out, lhsT.bitcast(F32R), rhs.bitcast(F32R),
                     start=start, stop=stop, skip_group_check=skip_group_check)


def build_program():
    nc = bacc.Bacc("TRN2", target_bir_lowering=False, debug=False,
                   num_devices=NCORES)

    def din(name, shape, dt=F32R):
        return nc.dram_tensor(name, shape, dt, kind="ExternalInput").ap()

    xT = din("xT", [D, T])
    encT = din("encT", [D, T])
    wq1 = din("wq1", [D, D]); wk1 = din("wk1", [D, D]); wv1 = din("wv1", [D, D])
    wo1 = din("wo1", [D, D]); bd1 = din("bd1", [D, HM])
    wq2 = din("wq2", [D, D]); wk2 = din("wk2", [D, D]); wv2 = din("wv2", [D, D])
    wo2 = din("wo2", [D, D]); bd2 = din("bd2", [D, HM])
    e16T_d = din("e16T", [HM, H]); e16_d = din("e16", [H, HM])
    kvmask_d = din("kvmask", [HM, D], F32)
    w1 = din("w1", [D, DFF]); w2 = din("w2", [DFF, D], BF16)
    b1r_d = din("b1r", [1, DFF]); b2r_d = din("b2r", [1, D])
    gbe_d = din("gbe", [128, 6 * DC], F32)  # g1|be1|g2|be2|g3|be3, chunk-packed
    ones_col_d = din("ones_col", [128, 8])
    ones_row_d = din("ones_row", [1, 128])
    ones_tb_d = din("ones_tb", [1, TB])

    outT = nc.dram_tensor("outT", [D, T], F32, kind="ExternalOutput").ap()

    with nc.allow_low_precision(reason="f32r matmul inputs (4-byte storage)"), \
         tile.TileContext(nc) as tc, ExitStack() as top:
        dram = top.enter_context(tc.tile_pool(name="dram", bufs=1, space="DRAM"))
        h_spill = dram.tile([DFF, T], BF16)
        out2_spill = dram.tile([D, T], F32R)
        arin1 = dram.tile([HM, B * (D + 1)], F32)
        arout1 = dram.tile([HM, B * (D + 1)], F32, addr_space="Shared")
        arin2 = dram.tile([HM, B * (D + 1)], F32)
        arout2 = dram.tile([HM, B * (D + 1)], F32, addr_space="Shared")

        const = top.enter_context(tc.tile_pool(name="const", bufs=1))
        e16T = const.tile([HM, H], F32R); nc.sync.dma_start(e16T[:], e16T_d[:])
        e16 = const.tile([H, HM], F32R); nc.sync.dma_start(e16[:], e16_d[:])
        gbe = const.tile([128, 6 * DC], F32); nc.sync.dma_start(gbe[:], gbe_d[:])
        ones_col = const.tile([128, 8], F32R); nc.sync.dma_start(ones_col[:], ones_col_d[:])
        ones_row = const.tile([1, 128], F32R); nc.sync.dma_start(ones_row[:], ones_row_d[:])
        ones_tb = const.tile([1, TB], F32R); nc.sync.dma_start(ones_tb[:], ones_tb_d[:])
        eps_t = const.tile([1, 1], F32); nc.vector.memset(eps_t[:], EPS_LN)

        def gslice(i):   # per-partition [128,1] scale slice for LN i (0,1,2)
            return gbe[:, 2 * i * DC:(2 * i + 1) * DC]

        def beslice(i):
            return gbe[:, (2 * i + 1) * DC:(2 * i + 2) * DC]

        # residual stream: per-batch [128, DC*TB] tiles, feature-major
        # layout: tile[p, kc*TB + t] = act[kc*128+p, b*TB+t]
        # `mid` closes before the W2 phase so its SBUF is released.
        mid = ExitStack()
        resid = mid.enter_context(tc.tile_pool(name="resid", bufs=5))
        qp_pool = mid.enter_context(tc.tile_pool(name="qp", bufs=1))

        def load_wide(pool, src_dram, ncols, name):
            """DRAM [D, ncols] -> SBUF [128, DC*ncols], chunk kc at cols kc*ncols."""
            t_ = pool.tile([128, DC * ncols], F32R, name=name)
            for kc in range(DC):
                nc.sync.dma_start(t_[:, kc * ncols:(kc + 1) * ncols],
                                  src_dram[kc * 128:(kc + 1) * 128, :])
            return t_

        # ---------------- P0: load x feature-major, per batch ----------------
        x_b = []
        for b in range(B):
            xb = resid.tile([128, DC * TB], F32R, tag="resid", name=f"x{b}")
            for kc in range(DC):
                nc.sync.dma_start(xb[:, kc * TB:(kc + 1) * TB],
                                  xT[kc * 128:(kc + 1) * 128, b * TB:(b + 1) * TB])
            x_b.append(xb)

        def kv_phase(wk_d, wv_d, bd_d, inp_b, arin, suffix):
            """K/V projections, kp features, kv-junk accumulation, AR input.

            Two passes so Wk and Wv are not co-resident: A) K + kp for all
            batches (kp kept, 8KB/part); B) V + kv-junk accumulation."""
            with ExitStack() as ph:
                kpp = ph.enter_context(tc.tile_pool(name=f"kpp{suffix}", bufs=16))
                kp_t = {}
                with ExitStack() as pa:
                    wp = pa.enter_context(tc.tile_pool(name=f"wkp{suffix}", bufs=1))
                    work = pa.enter_context(tc.tile_pool(name=f"kvw{suffix}", bufs=1))
                    gps = pa.enter_context(tc.tile_pool(name=f"gpsk{suffix}", bufs=3, space="PSUM"))
                    kps = pa.enter_context(tc.tile_pool(name=f"kps{suffix}", bufs=2, space="PSUM"))
                    wk_sb = load_wide(wp, wk_d, D, f"wk{suffix}")
                    bd_sb = load_wide(wp, bd_d, HM, f"bd{suffix}")
                    for b in range(B):
                        xb = inp_b[b]
                        kf = work.tile([128, DC * TB], F32R, tag="kf", name=f"kf{suffix}{b}")
                        for mc in range(DC):
                            ps = gps.tile([128, TB], F32, tag="g", name=f"kps_{suffix}")
                            for kc in range(DC):
                                _mm(nc, ps[:], wk_sb[:, kc * D + mc * 128: kc * D + mc * 128 + 128],
                                    xb[:, kc * TB:(kc + 1) * TB], kc == 0, kc == DC - 1)
                            nc.any.tensor_copy(kf[:, mc * TB:(mc + 1) * TB], ps[:])
                        for ts in range(TB // 128):
                            kpt = kpp.tile([128, HM], F32R, tag="kp", name=f"kp{suffix}_{b}_{ts}")
                            psk = kps.tile([128, HM], F32, tag="kpps", name=f"kpps{suffix}")
                            for kc in range(DC):
                                _mm(nc, psk[:],
                                    kf[:, kc * TB + ts * 128: kc * TB + ts * 128 + 128],
                                    bd_sb[:, kc * HM: (kc + 1) * HM],
                                    kc == 0, kc == DC - 1)
                            nc.vector.tensor_scalar(kpt[:], psk[:], 0.0, STAB,
                                                    AluOpType.max, AluOpType.add)
                            kp_t[(b, ts)] = kpt

                with ExitStack() as pb:
                    wp = pb.enter_context(tc.tile_pool(name=f"wvp{suffix}", bufs=1))
                    vp = pb.enter_context(tc.tile_pool(name=f"vtp{suffix}", bufs=3))
                    kvo = pb.enter_context(tc.tile_pool(name=f"kvo{suffix}", bufs=2))
                    gps = pb.enter_context(tc.tile_pool(name=f"gpsv{suffix}", bufs=3, space="PSUM"))
                    kvps = pb.enter_context(tc.tile_pool(name=f"kvps{suffix}", bufs=1, space="PSUM"))
                    wv_sb = load_wide(wp, wv_d, D, f"wv{suffix}")
                    for b in range(B):
                        xb = inp_b[b]
                        kvjA = kvps.tile([128, 512], F32, tag="A", name=f"kvjA{suffix}")
                        kvjB = kvps.tile([128, 512], F32, tag="Bt", name=f"kvjB{suffix}")
                        kvjS = kvps.tile([128, 8], F32, tag="S", name=f"kvjS{suffix}")
                        for ts in range(TB // 128):
                            vt = vp.tile([128, D], F32R, tag="vt", name=f"vt{suffix}")
                            for nb in range(2):
                                ps = gps.tile([128, 512], F32, tag="g", name=f"vps_{suffix}")
                                for kc in range(DC):
                                    _mm(nc, ps[:],
                                        xb[:, kc * TB + ts * 128: kc * TB + ts * 128 + 128],
                                        wv_sb[:, kc * D + nb * 512: kc * D + nb * 512 + 512],
                                        kc == 0, kc == DC - 1)
                                nc.any.tensor_copy(vt[:, nb * 512:(nb + 1) * 512], ps[:])

                            kpt = kp_t[(b, ts)]
                            first, last = ts == 0, ts == TB // 128 - 1
                            _mm(nc, kvjA[:], kpt[:], vt[:, 0:512], first, last, True)
                            _mm(nc, kvjB[:], kpt[:], vt[:, 512:1024], first, last, True)
                            _mm(nc, kvjS[:], kpt[:], ones_col[:], first, last, True)

                        kvj = kvo.tile([128, D + 1], F32, tag="kvj", name=f"kvj{suffix}")
                        nc.any.tensor_copy(kvj[:, 0:512], kvjA[:])
                        nc.any.tensor_copy(kvj[:, 512:1024], kvjB[:])
                        nc.any.tensor_copy(kvj[:, 1024:1025], kvjS[:, 0:1])
                        nc.sync.dma_start(arin[:, b * (D + 1):(b + 1) * (D + 1)], kvj[:])

        def q_phase(wq_d, bd_d, inp_b, qp_fm, suffix, from_dram=None):
            """Q projection + qp features -> qp_fm [128, T] feature-major."""
            with ExitStack() as ph:
                wp = ph.enter_context(tc.tile_pool(name=f"wq{suffix}", bufs=1))
                work = ph.enter_context(tc.tile_pool(name=f"qw{suffix}", bufs=2))
                gps = ph.enter_context(tc.tile_pool(name=f"qgps{suffix}", bufs=2, space="PSUM"))
                qps_ = ph.enter_context(tc.tile_pool(name=f"qpps{suffix}", bufs=2, space="PSUM"))

                wq_sb = load_wide(wp, wq_d, D, f"wqw{suffix}")
                bd_sb = load_wide(wp, bd_d, HM, f"bdq{suffix}")

                for b in range(B):
                    if from_dram is not None:
                        xb = work.tile([128, DC * TB], F32R, tag="encb", name=f"encb{suffix}")
                        for kc in range(DC):
                            nc.sync.dma_start(
                                xb[:, kc * TB:(kc + 1) * TB],
                                from_dram[kc * 128:(kc + 1) * 128, b * TB:(b + 1) * TB])
                    else:
                        xb = inp_b[b]
                    qf = work.tile([128, DC * TB], F32R, tag="qf", name=f"qf{suffix}", bufs=1)
                    for mc in range(DC):
                        ps = gps.tile([128, TB], F32, tag="g", name=f"qps_{suffix}")
                        for kc in range(DC):
                            _mm(nc, ps[:], wq_sb[:, kc * D + mc * 128: kc * D + mc * 128 + 128],
                                xb[:, kc * TB:(kc + 1) * TB], kc == 0, kc == DC - 1)
                        nc.any.tensor_copy(qf[:, mc * TB:(mc + 1) * TB], ps[:])
                    pq = qps_.tile([128, TB], F32, tag="qp", name=f"qpps_{suffix}")
                    for kc in range(DC):
                        _mm(nc, pq[:], bd_sb[:, kc * HM:(kc + 1) * HM],
                            qf[:, kc * TB:(kc + 1) * TB], kc == 0, kc == DC - 1)
                    nc.vector.tensor_scalar(qp_fm[:, b * TB:(b + 1) * TB], pq[:],
                                            0.0, STAB, AluOpType.max, AluOpType.add)

        def favor_out_phase(wo_d, arout, qp_fm, inp_b, out_b_list, ln_idx, suffix,
                            spill_to=None):
            """o = (qp/z) @ kv per head, o-proj, residual + LN -> out_b tiles."""
            with ExitStack() as ph:
                wp = ph.enter_context(tc.tile_pool(name=f"wo{suffix}", bufs=1))
                kvp = ph.enter_context(tc.tile_pool(name=f"kvi{suffix}", bufs=2))
                bdkvp = ph.enter_context(tc.tile_pool(name=f"bdkv{suffix}", bufs=1))
                fv = ph.enter_context(tc.tile_pool(name=f"fv{suffix}", bufs=1))
                ofm = ph.enter_context(tc.tile_pool(name=f"ofm{suffix}", bufs=1))
                r1p = ph.enter_context(tc.tile_pool(name=f"r1{suffix}", bufs=1))
                sqp = ph.enter_context(tc.tile_pool(name=f"sq{suffix}", bufs=2))
                stp = ph.enter_context(tc.tile_pool(name=f"st{suffix}", bufs=8))
                gps = ph.enter_context(tc.tile_pool(name=f"ogps{suffix}", bufs=3, space="PSUM"))
                sps = ph.enter_context(tc.tile_pool(name=f"osps{suffix}", bufs=5, space="PSUM"))

                wo_sb = load_wide(wp, wo_d, D, f"wow{suffix}")
                kvmask = kvp.tile([HM, D], F32, tag="kvmask", name=f"kvmask{suffix}")
                nc.sync.dma_start(kvmask[:], kvmask_d[:])

                for b in range(B):
                    bs = b * (D + 1)
                    kvb = kvp.tile([HM, D + 1], F32, tag="kvb", name=f"kvb{suffix}")
                    nc.sync.dma_start(kvb[:], arout[:, bs:bs + D + 1])
                    bdkv = bdkvp.tile([HM, D], F32R, tag="bdkv", name=f"bdkv_{suffix}")
                    nc.vector.tensor_tensor(bdkv[:], kvb[:, 0:D], kvmask[:],
                                            AluOpType.mult)
                    # z = e16T^T @ (qp * ksum_col) ; per-partition scalar = ksum
                    qpk = fv.tile([128, TB], F32R, tag="qpk", name=f"qpk{suffix}")
                    nc.vector.tensor_scalar(qpk[:], qp_fm[:, b * TB:(b + 1) * TB],
                                            kvb[:, D:D + 1], None,
                                            AluOpType.mult)
                    zps = sps.tile([H, TB], F32, tag="s", name=f"z{suffix}")
                    _mm(nc, zps[:], e16T[:], qpk[:], True, True)
                    rz = fv.tile([H, TB], F32R, tag="rz", name=f"rz{suffix}")
                    nc.vector.reciprocal(rz[:], zps[:])
                    if NEWTON:
                        t1 = fv.tile([H, TB], F32, tag="nt1", name=f"nt1{suffix}")
                        nc.vector.tensor_tensor(t1[:], zps[:], rz[:], AluOpType.mult)
                        nc.vector.tensor_scalar(t1[:], t1[:], -1.0, 2.0,
                                                AluOpType.mult, AluOpType.add)
                        nc.vector.tensor_tensor(rz[:], rz[:], t1[:], AluOpType.mult)
                    zbc = sps.tile([128, TB], F32, tag="s", name=f"zbc{suffix}")
                    _mm(nc, zbc[:], e16[:], rz[:], True, True)
                    qps_t = fv.tile([128, TB], F32R, tag="qps", name=f"qps{suffix}")
                    nc.vector.tensor_tensor(qps_t[:], qp_fm[:, b * TB:(b + 1) * TB],
                                            zbc[:], AluOpType.mult)

                    # o feature-major via block-diag kv
                    of = ofm.tile([128, DC * TB], F32R, tag="of", name=f"of{suffix}")
                    for c in range(DC):
                        ps = gps.tile([128, TB], F32, tag="g", name=f"ops_{suffix}")
                        _mm(nc, ps[:], bdkv[:, c * 128:(c + 1) * 128], qps_t[:],
                            True, True)
                        nc.any.tensor_copy(of[:, c * TB:(c + 1) * TB], ps[:])

                    # o-proj + residual + LN stats
                    r1 = r1p.tile([128, DC * TB], F32R, tag="r1", name=f"r1{suffix}")
                    Sp = sps.tile([1, TB], F32, tag="s", name=f"S{suffix}")
                    SSp = sps.tile([1, TB], F32, tag="s", name=f"SS{suffix}")
                    for mc in range(DC):
                        ps = gps.tile([128, TB], F32, tag="g", name=f"ojps_{suffix}")
                        for kc in range(DC):
                            _mm(nc, ps[:], wo_sb[:, kc * D + mc * 128: kc * D + mc * 128 + 128],
                                of[:, kc * TB:(kc + 1) * TB], kc == 0, kc == DC - 1)
                        nc.vector.tensor_tensor(r1[:, mc * TB:(mc + 1) * TB], ps[:],
                                                inp_b[b][:, mc * TB:(mc + 1) * TB],
                                                AluOpType.add)
                        sq = sqp.tile([128, TB], F32R, tag="sq", name=f"sq{suffix}")
                        nc.scalar.activation(sq[:], r1[:, mc * TB:(mc + 1) * TB], AF.Square)
                        _mm(nc, Sp[:], ones_col[:, 0:1], r1[:, mc * TB:(mc + 1) * TB],
                            mc == 0, mc == DC - 1, True)
                        _mm(nc, SSp[:], ones_col[:, 0:1], sq[:], mc == 0, mc == DC - 1, True)

                    # stats -> a (rstd), bb (-m*rstd)
                    mneg = stp.tile([1, TB], F32, tag="st", name=f"mneg{suffix}")
                    nc.vector.tensor_scalar(mneg[:], Sp[:], -1.0 / D, None, AluOpType.mult)
                    m2 = stp.tile([1, TB], F32, tag="st", name=f"m2{suffix}")
                    nc.vector.tensor_tensor(m2[:], mneg[:], mneg[:], AluOpType.mult)
                    ve = stp.tile([1, TB], F32, tag="st", name=f"ve{suffix}")
                    nc.vector.scalar_tensor_tensor(ve[:], in0=SSp[:], scalar=1.0 / D,
                                                   in1=m2[:], op0=AluOpType.mult,
                                                   op1=AluOpType.subtract)
                    sqv = stp.tile([1, TB], F32, tag="st", name=f"sqv{suffix}")
                    nc.scalar.activation(sqv[:], ve[:], AF.Sqrt, bias=eps_t[:])
                    a_ = stp.tile([1, TB], F32R, tag="st", name=f"a{suffix}")
                    nc.vector.reciprocal(a_[:], sqv[:])
                    if NEWTON:
                        n1 = stp.tile([1, TB], F32, tag="st", name=f"n1{suffix}")
                        nc.vector.tensor_tensor(n1[:], a_[:], a_[:], AluOpType.mult)
                        n2 = stp.tile([1, TB], F32, tag="st", name=f"n2{suffix}")
                        nc.vector.scalar_tensor_tensor(n2[:], in0=ve[:], scalar=EPS_LN,
                                                       in1=n1[:], op0=AluOpType.add,
                                                       op1=AluOpType.mult)
                        nc.vector.tensor_scalar(n2[:], n2[:], -0.5, 1.5,
                                                AluOpType.mult, AluOpType.add)
                        nc.vector.tensor_tensor(a_[:], a_[:], n2[:], AluOpType.mult)
                    bb = stp.tile([1, TB], F32R, tag="st", name=f"bb{suffix}")
                    nc.vector.tensor_tensor(bb[:], mneg[:], a_[:], AluOpType.mult)
                    abc = sps.tile([128, TB], F32, tag="s", name=f"abc{suffix}")
                    _mm(nc, abc[:], ones_row[:], a_[:], True, True)
                    bbc = sps.tile([128, TB], F32, tag="s", name=f"bbc{suffix}")
                    _mm(nc, bbc[:], ones_row[:], bb[:], True, True)

                    ob = resid.tile([128, DC * TB], F32R, tag="resid",
                                    name=f"out{ln_idx}_{b}")
                    for mc in range(DC):
                        tpm = sqp.tile([128, TB], F32, tag="sq", name=f"tpm{suffix}")
                        nc.vector.tensor_tensor(tpm[:], r1[:, mc * TB:(mc + 1) * TB],
                                                abc[:], AluOpType.mult)
                        nc.vector.tensor_tensor(tpm[:], tpm[:], bbc[:], AluOpType.add)
                        nc.scalar.activation(ob[:, mc * TB:(mc + 1) * TB], tpm[:],
                                             AF.Identity, bias=beslice(ln_idx)[:, mc:mc + 1],
                                             scale=gslice(ln_idx)[:, mc:mc + 1])
                    if spill_to is not None:
                        for kc in range(DC):
                            nc.sync.dma_start(
                                spill_to[kc * 128:(kc + 1) * 128, b * TB:(b + 1) * TB],
                                ob[:, kc * TB:(kc + 1) * TB])
                    out_b_list.append(ob)

        def allreduce(arin, arout):
            nc.gpsimd.collective_compute(
                "AllReduce", AluOpType.add,
                replica_groups=[list(range(NCORES))],
                ins=[arin[:]], outs=[arout[:]])

        # =================== attention 1 (self) ===================
        kv_phase(wk1, wv1, bd1, x_b, arin1, "a1")
        allreduce(arin1, arout1)
        qp1 = qp_pool.tile([HM, T], F32R, tag="qp", name="qp1")
        q_phase(wq1, bd1, x_b, qp1, "a1")
        out1_b = []
        favor_out_phase(wo1, arout1, qp1, x_b, out1_b, 0, "a1")

        # =================== attention 2 (cross: q from enc, kv from out1) ===
        kv_phase(wk2, wv2, bd2, out1_b, arin2, "a2")
        allreduce(arin2, arout2)
        qp2 = qp_pool.tile([HM, T], F32R, tag="qp", name="qp2")
        q_phase(wq2, bd2, None, qp2, "a2", from_dram=encT)
        out2_b = []
        favor_out_phase(wo2, arout2, qp2, out1_b, out2_b, 1, "a2",
                        spill_to=out2_spill)

        # =================== FFN ===================
        # P7a: h = elu(out2 @ W1 + b1), spilled to DRAM feature-major [DFF, T]
        with ExitStack() as ph:
            wp = ph.enter_context(tc.tile_pool(name="w1p", bufs=2))
            hp = ph.enter_context(tc.tile_pool(name="hp", bufs=3))
            ep = ph.enter_context(tc.tile_pool(name="ep", bufs=3))
            b1p = ph.enter_context(tc.tile_pool(name="b1p", bufs=1))
            hps = ph.enter_context(tc.tile_pool(name="hps", bufs=4, space="PSUM"))
            b1row = b1p.tile([1, DFF], F32R, name="b1row")
            nc.sync.dma_start(b1row[:], b1r_d[:])
            for dffc in range(DFF // 512):
                w1c = wp.tile([128, DC * 512], F32R, tag="w1c", name="w1c")
                for kc in range(DC):
                    nc.sync.dma_start(w1c[:, kc * 512:(kc + 1) * 512],
                                      w1[kc * 128:(kc + 1) * 128,
                                         dffc * 512:(dffc + 1) * 512])
                for b in range(B):
                    for ms in range(4):
                        ps = hps.tile([128, TB], F32, tag="h", name="hps_t")
                        for kc in range(DC):
                            _mm(nc, ps[:],
                                w1c[:, kc * 512 + ms * 128: kc * 512 + ms * 128 + 128],
                                out2_b[b][:, kc * TB:(kc + 1) * TB],
                                kc == 0, False)
                        _mm(nc, ps[:],
                            b1row[0:1, dffc * 512 + ms * 128: dffc * 512 + ms * 128 + 128],
                            ones_tb[:], False, True)
                        # ELU: h = min(exp(u) - 1, max(u, 0))
                        e_ = ep.tile([128, TB], F32, tag="e", name="e_t")
                        nc.scalar.activation(e_[:], ps[:], AF.Exp)
                        t_ = ep.tile([128, TB], F32, tag="t", name="t_t")
                        nc.vector.tensor_scalar(t_[:], ps[:], 0.0, None, AluOpType.max)
                        h_ = hp.tile([128, TB], BF16, tag="hsb", name="h_t")
                        nc.vector.scalar_tensor_tensor(h_[:], in0=e_[:], scalar=1.0,
                                                       in1=t_[:], op0=AluOpType.subtract,
                                                       op1=AluOpType.min)
                        nc.sync.dma_start(
                            h_spill[dffc * 512 + ms * 128: dffc * 512 + ms * 128 + 128,
                                    b * TB:(b + 1) * TB], h_[:])

        # P7b: r3 = h @ W2 + b2 + out2 ; LN3 -> outT
        # resid/qp pools close here; W2 takes their space.
        mid.close()
        TB3 = 512
        with ExitStack() as ph:
            wp = ph.enter_context(tc.tile_pool(name="w2p", bufs=1))
            b2p = ph.enter_context(tc.tile_pool(name="b2p", bufs=1))
            hin = ph.enter_context(tc.tile_pool(name="hin", bufs=4))
            o2p = ph.enter_context(tc.tile_pool(name="o2p", bufs=2))
            r3p = ph.enter_context(tc.tile_pool(name="r3p", bufs=1))
            sqp = ph.enter_context(tc.tile_pool(name="sq3", bufs=2))
            stp = ph.enter_context(tc.tile_pool(name="st3", bufs=8))
            o3p = ph.enter_context(tc.tile_pool(name="o3p", bufs=3))
            # one shared PSUM pool: 8 r3 banks rotate with the LN3 stat tiles
            rps = ph.enter_context(tc.tile_pool(name="rps", bufs=8, space="PSUM"))

            w2_sb = wp.tile([128, (DFF // 128) * D], BF16, name="w2sb")
            for kc in range(DFF // 128):
                nc.sync.dma_start(w2_sb[:, kc * D:(kc + 1) * D],
                                  w2[kc * 128:(kc + 1) * 128, :])
            b2row = b2p.tile([1, D], F32R, name="b2row")
            nc.sync.dma_start(b2row[:], b2r_d[:])

            for t3 in range(T // TB3):
                # one psum bank per d-chunk (start=True zeroes a whole bank)
                rt = [rps.tile([128, TB3], F32, tag="r3", name=f"r3ps{i}")
                      for i in range(DC)]
                for kc in range(DFF // 128):
                    hk = hin.tile([128, TB3], BF16, tag="hk", name="hk")
                    nc.sync.dma_start(hk[:], h_spill[kc * 128:(kc + 1) * 128,
                                                     t3 * TB3:(t3 + 1) * TB3])
                    for c in range(DC):
                        nc.tensor.matmul(rt[c][:],
                            w2_sb[:, kc * D + c * 128: kc * D + c * 128 + 128],
                            hk[:], start=(kc == 0), stop=False,
                            skip_group_check=True)
                # b2 row: finish accumulation groups
                for c in range(DC):
                    _mm(nc, rt[c][:],
                        b2row[0:1, c * 128:(c + 1) * 128],
                        ones_tb[0:1, 0:TB3], False, True, True)

                r3 = r3p.tile([128, DC * TB3], F32R, tag="r3s", name="r3s")
                Sp = rps.tile([1, TB3], F32, tag="r3", name="S3")
                SSp = rps.tile([1, TB3], F32, tag="r3", name="SS3")
                for c in range(DC):
                    o2c = o2p.tile([128, TB3], F32R, tag="o2c", name="o2c")
                    nc.sync.dma_start(o2c[:], out2_spill[c * 128:(c + 1) * 128,
                                                         t3 * TB3:(t3 + 1) * TB3])
                    nc.vector.tensor_tensor(r3[:, c * TB3:(c + 1) * TB3], rt[c][:],
                                            o2c[:], AluOpType.add)
                    sq = sqp.tile([128, TB3], F32R, tag="sq3", name="sq3t")
                    nc.scalar.activation(sq[:], r3[:, c * TB3:(c + 1) * TB3], AF.Square)
                    _mm(nc, Sp[:], ones_col[:, 0:1], r3[:, c * TB3:(c + 1) * TB3],
                        c == 0, c == DC - 1, True)
                    _mm(nc, SSp[:], ones_col[:, 0:1], sq[:], c == 0, c == DC - 1, True)

                mneg = stp.tile([1, TB3], F32, tag="st3", name="mneg3")
                nc.vector.tensor_scalar(mneg[:], Sp[:], -1.0 / D, None, AluOpType.mult)
                m2 = stp.tile([1, TB3], F32, tag="st3", name="m23")
                nc.vector.tensor_tensor(m2[:], mneg[:], mneg[:], AluOpType.mult)
                ve = stp.tile([1, TB3], F32, tag="st3", name="ve3")
                nc.vector.scalar_tensor_tensor(ve[:], in0=SSp[:], scalar=1.0 / D,
                                               in1=m2[:], op0=AluOpType.mult,
                                               op1=AluOpType.subtract)
                sqv = stp.tile([1, TB3], F32, tag="st3", name="sqv3")
                nc.scalar.activation(sqv[:], ve[:], AF.Sqrt, bias=eps_t[:])
                a_ = stp.tile([1, TB3], F32R, tag="st3", name="a3")
                nc.vector.reciprocal(a_[:], sqv[:])
                if NEWTON:
                    n1 = stp.tile([1, TB3], F32, tag="st3", name="n13")
                    nc.vector.tensor_tensor(n1[:], a_[:], a_[:], AluOpType.mult)
                    n2 = stp.tile([1, TB3], F32, tag="st3", name="n23")
                    nc.vector.scalar_tensor_tensor(n2[:], in0=ve[:], scalar=EPS_LN,
                                                   in1=n1[:], op0=AluOpType.add,
                                                   op1=AluOpType.mult)
                    nc.vector.tensor_scalar(n2[:], n2[:], -0.5, 1.5,
                                            AluOpType.mult, AluOpType.add)
                    nc.vector.tensor_tensor(a_[:], a_[:], n2[:], AluOpType.mult)
                bb = stp.tile([1, TB3], F32R, tag="st3", name="bb3")
                nc.vector.tensor_tensor(bb[:], mneg[:], a_[:], AluOpType.mult)
                abc = rps.tile([128, TB3], F32, tag="r3", name="abc3")
                _mm(nc, abc[:], ones_row[:], a_[:], True, True)
                bbc = rps.tile([128, TB3], F32, tag="r3", name="bbc3")
                _mm(nc, bbc[:], ones_row[:], bb[:], True, True)

                for c in range(DC):
                    tpm = sqp.tile([128, TB3], F32, tag="sq3", name="tpm3")
                    nc.vector.tensor_tensor(tpm[:], r3[:, c * TB3:(c + 1) * TB3],
                                            abc[:], AluOpType.mult)
                    nc.vector.tensor_tensor(tpm[:], tpm[:], bbc[:], AluOpType.add)
                    o3 = o3p.tile([128, TB3], F32, tag="o3", name="o3t")
                    nc.scalar.activation(o3[:], tpm[:], AF.Identity,
                                         bias=beslice(2)[:, c:c + 1],
                                         scale=gslice(2)[:, c:c + 1])
                    nc.sync.dma_start(outT[c * 128:(c + 1) * 128,
                                           t3 * TB3:(t3 + 1) * TB3], o3[:])

    nc.compile()
    return nc


def _host_prep(inputs):
    """Build per-core in_maps from full inputs."""
    f32 = np.float32
    x = np.asarray(inputs['x'], f32)
    enc = np.asarray(inputs['enc_output'], f32)

    def bdiag(P):
        bd = np.zeros((D, HM), f32)
        pt = (np.asarray(P, f32) / np.sqrt(M)).T  # [DH, M]
        for h in range(H):
            bd[h * DH:(h + 1) * DH, h * M:(h + 1) * M] = pt
        return bd

    e16T = np.zeros((HM, H), f32)
    e16 = np.zeros((H, HM), f32)
    kvmask = np.zeros((HM, D), f32)
    for h in range(H):
        e16T[h * M:(h + 1) * M, h] = 1.0
        e16[h, h * M:(h + 1) * M] = 1.0
        kvmask[h * M:(h + 1) * M, h * DH:(h + 1) * DH] = 1.0

    gbe = np.zeros((128, 6 * DC), f32)
    for i, nm in enumerate(['g1', 'be1', 'g2', 'be2', 'g3', 'be3']):
        gbe[:, i * DC:(i + 1) * DC] = np.asarray(inputs[nm], f32).reshape(DC, 128).T

    shared = {
        'wq1': np.ascontiguousarray(np.asarray(inputs['Wq1'], f32).reshape(D, D)),
        'wk1': np.ascontiguousarray(np.asarray(inputs['Wk1'], f32).reshape(D, D)),
        'wv1': np.ascontiguousarray(np.asarray(inputs['Wv1'], f32).reshape(D, D)),
        'wo1': np.ascontiguousarray(np.asarray(inputs['Wo1'], f32).reshape(D, D)),
        'bd1': bdiag(inputs['P1']),
        'wq2': np.ascontiguousarray(np.asarray(inputs['Wq2'], f32).reshape(D, D)),
        'wk2': np.ascontiguousarray(np.asarray(inputs['Wk2'], f32).reshape(D, D)),
        'wv2': np.ascontiguousarray(np.asarray(inputs['Wv2'], f32).reshape(D, D)),
        'wo2': np.ascontiguousarray(np.asarray(inputs['Wo2'], f32).reshape(D, D)),
        'bd2': bdiag(inputs['P2']),
        'e16T': e16T, 'e16': e16, 'kvmask': kvmask,
        'w1': np.ascontiguousarray(np.asarray(inputs['W1'], f32)),
        'w2': np.ascontiguousarray(np.asarray(inputs['W2'], f32)).astype(ml_dtypes.bfloat16),
        'b1r': np.asarray(inputs['b1'], f32).reshape(1, DFF),
        'b2r': np.asarray(inputs['b2'], f32).reshape(1, D),
        'gbe': gbe,
        'ones_col': np.ones((128, 8), f32),
        'ones_row': np.ones((1, 128), f32),
        'ones_tb': np.ones((1, TB), f32),
    }

    in_maps = []
    for i in range(NCORES):
        sl = slice(i * LSH, (i + 1) * LSH)
        m = dict(shared)
        m['xT'] = np.ascontiguousarray(
            x[:, sl, :].transpose(2, 0, 1).reshape(D, T))
        m['encT'] = np.ascontiguousarray(
            enc[:, sl, :].transpose(2, 0, 1).reshape(D, T))
        in_maps.append(m)
    return in_maps


def kernel(**inputs) -> np.ndarray:
    if 'nc' not in _cache:
        _cache['nc'] = build_program()
    nc = _cache['nc']
    in_maps = _host_prep(inputs)
    res = run_bass_kernel_spmd(nc, in_maps, core_ids=list(range(NCORES)))
    out = np.empty((B, L, D), np.float32)
    for i in range(NCORES):
        o = res.results[i]['outT']  # [D, T] feature-major
        out[:, i * LSH:(i + 1) * LSH, :] = o.reshape(D, B, LSH).transpose(1, 2, 0)
    return out


if __name__ == '__main__':
    np.random.seed(0)
    print("building program...")
    build_program()
    print("OK")



# revision 2
# speedup vs baseline: 1584.0094x; 1584.0094x over previous
"""Trainium2 Bass kernel for nn_DecoderLayer (Performer/FAVOR+ decoder layer).

v2: folded-projection FAVOR+. Because the Performer uses only M=8 random
features per head (HM = H*M = 128 total), the Q/K projections fold into the
random-feature projection on the host: wqe = Wq @ P^T/sqrt(M) is [D, 128], so
qp = relu(x @ wqe) + stab needs no D x D matmul. The V and O projections fold
through the kv statistic: kv = kp^T V = (kp^T X) Wv and the output
o @ Wo = qps @ ((kv blockdiag-masked) @ Wo), so the only D x D work left is
applied to the tiny [D x 128] ST statistic instead of the full sequence.

Sharding: sequence (L) split across 8 cores; the per-batch global statistics
ST = [X^T kp | kp-sums] (~2MB total for B=4) are AllReduced twice.
Residual stream is feature-major; the final LN3 runs token-major so the
output DMAs out in natural [T, D] layout (no host post-transpose).
"""
import sys
import os

sys.path.insert(0, '/opt/trn_rl_repo')

import numpy as np
import ml_dtypes
from contextlib import ExitStack

from concourse import bass, bacc, tile
import concourse.mybir as mybir
from concourse.bass_utils import run_bass_kernel_spmd
from concourse.alu_op_type import AluOpType

F32 = mybir.dt.float32
F32R = mybir.dt.float32r
BF16 = mybir.dt.bfloat16
AF = mybir.ActivationFunctionType
AX = mybir.AxisListType

B, L, D, H, DH, M, DFF = 4, 4096, 1024, 16, 64, 8, 4096
NCORES = 8
LSH = L // NCORES          # 512 tokens of L per core
T = B * LSH                # 2048 tokens per core
TB = LSH                   # tokens per batch element per core (512)
NTS = TB // 128            # 4 x 128-token blocks per batch
DC = D // 128              # 8 d-chunks
DFC = DFF // 128           # 32 dff-chunks
HM = H * M                 # 128 random features total
EPS_LN = 1e-6
STAB = 0.001
ARW = DC * 512 + 4         # allreduce width: ST (8 chunks x 4 batches x 128) + ksums

_cache = {}


def _mm(nc, out, lhsT, rhs, start, stop, skip=False):
    nc.tensor.matmul(out, lhsT.bitcast(F32R), rhs.bitcast(F32R),
                     start=start, stop=stop, skip_group_check=skip)


def _mmb(nc, out, lhsT, rhs, start, stop, skip=False):
    nc.tensor.matmul(out, lhsT, rhs, start=start, stop=stop,
                     skip_group_check=skip)


def build_program(loop=1):
    nc = bacc.Bacc("TRN2", target_bir_lowering=False, debug=False,
                   num_devices=NCORES)

    def din(name, shape, dt=F32R):
        return nc.dram_tensor(name, shape, dt, kind="ExternalInput").ap()

    tensors = dict(
        xT=din("xT", [D, T]),
        encT=din("encT", [D, T]),
        wqe1=din("wqe1", [D, HM]), wke1=din("wke1", [D, HM]),
        wqe2=din("wqe2", [D, HM]), wke2=din("wke2", [D, HM]),
        wv1=din("wv1", [D, D], BF16), wo1=din("wo1", [D, D], BF16),
        wv2=din("wv2", [D, D], BF16), wo2=din("wo2", [D, D], BF16),
        w1=din("w1", [D, DFF], BF16), w2=din("w2", [DFF, D], BF16),
        kvm4=din("kvm4", [D, HM], BF16),
        e16T_d=din("e16T", [HM, H]), e16_d=din("e16", [H, HM]),
        gbe_d=din("gbe", [128, 4 * DC], F32),
        b1c_d=din("b1c", [128, DFC], F32),
        b2r_d=din("b2r", [1, D]),
        g3r_d=din("g3r", [1, D]), be3r_d=din("be3r", [1, D]),
        ident_d=din("ident", [128, 128]),
        identb_d=din("identb", [128, 128], BF16),
        ones_col_d=din("ones_col", [128, 8]),
        ones_row_d=din("ones_row", [1, 128]),
    )

    out_d = nc.dram_tensor("out", [T, D], F32, kind="ExternalOutput").ap()

    with nc.allow_low_precision(reason="f32r/bf16 matmul inputs"), \
         tile.TileContext(nc) as tc, ExitStack() as top:
        dram = top.enter_context(tc.tile_pool(name="dram", bufs=1, space="DRAM"))

        const = top.enter_context(tc.tile_pool(name="const", bufs=1))
        c = {}
        c['e16T'] = const.tile([HM, H], F32R, name="e16T")
        nc.sync.dma_start(c['e16T'][:], tensors['e16T_d'][:])
        c['e16'] = const.tile([H, HM], F32R, name="e16")
        nc.sync.dma_start(c['e16'][:], tensors['e16_d'][:])
        c['gbe'] = const.tile([128, 4 * DC], F32, name="gbe")
        nc.sync.dma_start(c['gbe'][:], tensors['gbe_d'][:])
        c['b1c'] = const.tile([128, DFC], F32, name="b1c")
        nc.sync.dma_start(c['b1c'][:], tensors['b1c_d'][:])
        c['b2r'] = const.tile([1, D], F32R, name="b2r")
        nc.sync.dma_start(c['b2r'][:], tensors['b2r_d'][:])
        c['ident'] = const.tile([128, 128], F32R, name="ident")
        nc.sync.dma_start(c['ident'][:], tensors['ident_d'][:])
        c['identb'] = const.tile([128, 128], BF16, name="identb")
        nc.sync.dma_start(c['identb'][:], tensors['identb_d'][:])
        c['ones_col'] = const.tile([128, 8], F32R, name="ones_col")
        nc.sync.dma_start(c['ones_col'][:], tensors['ones_col_d'][:])
        c['ones_row'] = const.tile([1, 128], F32R, name="ones_row")
        nc.sync.dma_start(c['ones_row'][:], tensors['ones_row_d'][:])
        c['eps_t'] = const.tile([1, 1], F32, name="eps_t")
        nc.vector.memset(c['eps_t'][:], EPS_LN)
        c['eps_c'] = const.tile([128, 1], F32, name="eps_c")
        nc.vector.memset(c['eps_c'][:], EPS_LN)
        g3r = const.tile([1, D], F32R, name="g3r")
        nc.sync.dma_start(g3r[:], tensors['g3r_d'][:])
        be3r = const.tile([1, D], F32R, name="be3r")
        nc.sync.dma_start(be3r[:], tensors['be3r_d'][:])
        c['g3bc'] = const.tile([128, D], F32R, name="g3bc")
        c['be3bc'] = const.tile([128, D], F32R, name="be3bc")
        with tc.tile_pool(name="bc_ps", bufs=2, space="PSUM") as bcp:
            for src, dst in ((g3r, c['g3bc']), (be3r, c['be3bc'])):
                for half in range(2):
                    pg = bcp.tile([128, D // 2], F32, tag="bc", name="pbc")
                    _mm(nc, pg[:], c['ones_row'][:],
                        src[:, half * 512:(half + 1) * 512], True, True)
                    nc.any.tensor_copy(dst[:, half * 512:(half + 1) * 512], pg[:])

        for it in range(loop):
            build_iter(nc, tc, f"i{it}" if loop > 1 else "", tensors, c,
                       dram, out_d)

    nc.compile()
    return nc


def build_iter(nc, tc, sfx, tensors, c, dram, out_d):
    e16T, e16 = c['e16T'], c['e16']
    gbe, b1c, b2r = c['gbe'], c['b1c'], c['b2r']
    ident, identb = c['ident'], c['identb']
    ones_col, ones_row = c['ones_col'], c['ones_row']
    eps_t, g3bc, be3bc = c['eps_t'], c['g3bc'], c['be3bc']
    eps_c = c['eps_c']

    arin1 = dram.tile([128, ARW], F32, name=f"arin1{sfx}")
    arout1 = dram.tile([128, ARW], F32, addr_space="Shared", name=f"arout1{sfx}")
    arin2 = dram.tile([128, ARW], F32, name=f"arin2{sfx}")
    arout2 = dram.tile([128, ARW], F32, addr_space="Shared", name=f"arout2{sfx}")
    h_spill = dram.tile([DFF, T], BF16, name=f"hspill{sfx}")

    def gslice(i):
        return gbe[:, 2 * i * DC:(2 * i + 1) * DC]

    def beslice(i):
        return gbe[:, (2 * i + 1) * DC:(2 * i + 2) * DC]

    def load_wide(pool, src_dram, ncols, name, dt=F32R):
        nchunk = src_dram.shape[0] // 128
        t_ = pool.tile([128, nchunk * ncols], dt, name=name)
        for kc in range(nchunk):
            nc.sync.dma_start(t_[:, kc * ncols:(kc + 1) * ncols],
                              src_dram[kc * 128:(kc + 1) * 128, :])
        return t_

    def feat_phase(inp_b, wqe, wke, qp_fm, arin, suffix):
        """qp/kp features + ST statistic + ksum -> arin (DRAM)."""
        with ExitStack() as ph:
            work = ph.enter_context(tc.tile_pool(name=f"ftw{suffix}", bufs=2))
            kptp = ph.enter_context(tc.tile_pool(name=f"kptp{suffix}", bufs=2))
            arp = ph.enter_context(tc.tile_pool(name=f"arp{suffix}", bufs=1))
            pqk = ph.enter_context(tc.tile_pool(name=f"pqk{suffix}", bufs=2, space="PSUM"))
            tpp = ph.enter_context(tc.tile_pool(name=f"tpp{suffix}", bufs=4, space="PSUM"))
            stps = ph.enter_context(tc.tile_pool(name=f"stps{suffix}", bufs=2, space="PSUM"))

            arin_sb = arp.tile([128, ARW], F32, name=f"arin_sb{suffix}")
            for b in range(B):
                xb = inp_b[b]
                if qp_fm is not None:
                    pq = pqk.tile([128, TB], F32, tag="pqk", name=f"pq{suffix}")
                    for kc in range(DC):
                        _mm(nc, pq[:], wqe[:, kc * HM:(kc + 1) * HM],
                            xb[:, kc * TB:(kc + 1) * TB], kc == 0, kc == DC - 1)
                    nc.vector.tensor_scalar(qp_fm[:, b * TB:(b + 1) * TB], pq[:],
                                            0.0, STAB, AluOpType.max, AluOpType.add)
                pk = pqk.tile([128, TB], F32, tag="pqk", name=f"pk{suffix}")
                for kc in range(DC):
                    _mm(nc, pk[:], wke[:, kc * HM:(kc + 1) * HM],
                        xb[:, kc * TB:(kc + 1) * TB], kc == 0, kc == DC - 1)
                kpf = work.tile([128, TB], F32R, tag="kpf", name=f"kpf{suffix}")
                nc.vector.tensor_scalar(kpf[:], pk[:], 0.0, STAB,
                                        AluOpType.max, AluOpType.add)
                # global k-feature sum for z (per-batch column of arin)
                nc.vector.tensor_reduce(arin_sb[:, DC * 512 + b:DC * 512 + b + 1],
                                        kpf[:], AX.X, AluOpType.add)
                # kp token-major [128 tok, HM] per 128-token block
                kptm = kptp.tile([128, NTS * 128], F32R, tag="kptm",
                                 name=f"kptm{suffix}")
                for ts in range(NTS):
                    ptt = tpp.tile([128, 128], F32R, tag="tp", name=f"ptt{suffix}")
                    nc.tensor.transpose(ptt[:], kpf[:, ts * 128:(ts + 1) * 128],
                                        ident[:])
                    nc.any.tensor_copy(kptm[:, ts * 128:(ts + 1) * 128], ptt[:])
                # ST[d, hm] += x[l, d]*kp[l, hm]: transpose x chunks, accumulate
                stA = stps.tile([128, 512], F32, tag="st", name=f"stA{suffix}")
                stB = stps.tile([128, 512], F32, tag="st", name=f"stB{suffix}")
                for kc in range(DC):
                    stt = stA if kc < 4 else stB
                    col = (kc % 4) * 128
                    for ts in range(NTS):
                        xtt = tpp.tile([128, 128], F32R, tag="tp",
                                       name=f"xtt{suffix}")
                        nc.tensor.transpose(
                            xtt[:], xb[:, kc * TB + ts * 128:kc * TB + ts * 128 + 128],
                            ident[:])
                        xts = work.tile([128, 128], F32R, tag="xts",
                                        name=f"xts{suffix}")
                        nc.any.tensor_copy(xts[:], xtt[:])
                        _mm(nc, stt[:, col:col + 128], xts[:],
                            kptm[:, ts * 128:(ts + 1) * 128],
                            ts == 0, ts == NTS - 1, True)
                for kc in range(DC):
                    stt = stA if kc < 4 else stB
                    col = (kc % 4) * 128
                    nc.any.tensor_copy(
                        arin_sb[:, kc * 512 + b * 128:kc * 512 + b * 128 + 128],
                        stt[:, col:col + 128])
            nc.sync.dma_start(arin[:], arin_sb[:])

    def attn_out_phase(wv_d, wo_d, arout, qp_fm, inp_b, out_pool, out_tag,
                       out_dt, ln_idx, suffix):
        """kv reconstruction + o = qps @ kvW + residual + LN -> out tiles."""
        out_b_list = []
        with ExitStack() as ph:
            gps = ph.enter_context(tc.tile_pool(name=f"gps{suffix}", bufs=3, space="PSUM"))
            sps = ph.enter_context(tc.tile_pool(name=f"sps{suffix}", bufs=5, space="PSUM"))
            ksp = ph.enter_context(tc.tile_pool(name=f"ksp{suffix}", bufs=1))
            kvwp = ph.enter_context(tc.tile_pool(name=f"kvwp{suffix}", bufs=1))

            ksums = ksp.tile([128, 4], F32, name=f"ksums{suffix}")
            kvw = kvwp.tile([128, B * D], BF16, name=f"kvw{suffix}")
            with ExitStack() as s1:
                kvbp = s1.enter_context(tc.tile_pool(name=f"kvbp{suffix}", bufs=1))
                kvbd = kvbp.tile([128, DC * 512], BF16, name=f"kvbd{suffix}")
                with ExitStack() as s0:
                    stbp = s0.enter_context(tc.tile_pool(name=f"stbp{suffix}", bufs=1))
                    st_b = stbp.tile([128, DC * 512], BF16, name=f"stb{suffix}")
                    # load ST (f32 staging freed before weights arrive)
                    with ExitStack() as sg:
                        stg = sg.enter_context(tc.tile_pool(name=f"stg{suffix}", bufs=1))
                        st_f = stg.tile([128, ARW], F32, name=f"stf{suffix}")
                        nc.sync.dma_start(st_f[:], arout[:])
                        nc.any.tensor_copy(ksums[:],
                                           st_f[:, DC * 512:DC * 512 + 4])
                        nc.any.tensor_copy(st_b[:], st_f[:, 0:DC * 512])
                    # stage 1: kvT (masked per batch) with Wv resident
                    with ExitStack() as sv:
                        wvp = sv.enter_context(tc.tile_pool(name=f"wvp{suffix}", bufs=1))
                        wv_sb = load_wide(wvp, wv_d, D, f"wv{suffix}", BF16)
                        kvm_sb = load_wide(wvp, tensors['kvm4'], HM,
                                           f"kvm{suffix}", BF16)
                        for kc in range(DC):
                            pkv = gps.tile([128, 512], F32, tag="g",
                                           name=f"pkv{suffix}")
                            for dc in range(DC):
                                _mmb(nc, pkv[:],
                                     wv_sb[:, dc * D + kc * 128:dc * D + kc * 128 + 128],
                                     st_b[:, dc * 512:(dc + 1) * 512],
                                     dc == 0, dc == DC - 1)
                            for b in range(B):
                                nc.vector.tensor_tensor(
                                    kvbd[:, kc * 512 + b * 128:kc * 512 + b * 128 + 128],
                                    pkv[:, b * 128:(b + 1) * 128],
                                    kvm_sb[:, kc * HM:(kc + 1) * HM],
                                    AluOpType.mult)
                # stage 2: kvW = kvbd^T @ Wo per batch, with Wo resident
                with ExitStack() as s2:
                    wop = s2.enter_context(tc.tile_pool(name=f"wop{suffix}", bufs=1))
                    wo_sb = load_wide(wop, wo_d, D, f"wo{suffix}", BF16)
                    for b in range(B):
                        for half in range(2):
                            pw = gps.tile([128, 512], F32, tag="g",
                                          name=f"pw{suffix}")
                            for kc in range(DC):
                                _mmb(nc, pw[:],
                                     kvbd[:, kc * 512 + b * 128:kc * 512 + b * 128 + 128],
                                     wo_sb[:, kc * D + half * 512:kc * D + half * 512 + 512],
                                     kc == 0, kc == DC - 1)
                            nc.any.tensor_copy(
                                kvw[:, b * D + half * 512:b * D + half * 512 + 512],
                                pw[:])

            # stage 3: z, o, residual, LN per batch
            fv = ph.enter_context(tc.tile_pool(name=f"fv{suffix}", bufs=1))
            r1p = ph.enter_context(tc.tile_pool(name=f"r1{suffix}", bufs=1))
            sqp = ph.enter_context(tc.tile_pool(name=f"sq{suffix}", bufs=2))
            stp = ph.enter_context(tc.tile_pool(name=f"stt{suffix}", bufs=1))
            for b in range(B):
                qpk = fv.tile([128, TB], F32R, tag="qpk", name=f"qpk{suffix}")
                nc.vector.tensor_scalar(qpk[:], qp_fm[:, b * TB:(b + 1) * TB],
                                        ksums[:, b:b + 1], None, AluOpType.mult)
                zps = sps.tile([H, TB], F32, tag="s", name=f"z{suffix}")
                _mm(nc, zps[:], e16T[:], qpk[:], True, True)
                rz = fv.tile([H, TB], F32R, tag="rz", name=f"rz{suffix}")
                nc.vector.reciprocal(rz[:], zps[:])
                t1 = fv.tile([H, TB], F32, tag="nt1", name=f"nt1{suffix}")
                nc.vector.tensor_tensor(t1[:], zps[:], rz[:], AluOpType.mult)
                nc.vector.tensor_scalar(t1[:], t1[:], -1.0, 2.0,
                                        AluOpType.mult, AluOpType.add)
                nc.vector.tensor_tensor(rz[:], rz[:], t1[:], AluOpType.mult)
                zbc = sps.tile([128, TB], F32, tag="s", name=f"zbc{suffix}")
                _mm(nc, zbc[:], e16[:], rz[:], True, True)
                qps_t = fv.tile([128, TB], BF16, tag="qps", name=f"qps{suffix}")
                nc.vector.tensor_tensor(qps_t[:], qp_fm[:, b * TB:(b + 1) * TB],
                                        zbc[:], AluOpType.mult)

                r1 = r1p.tile([128, DC * TB], F32R, tag="r1", name=f"r1{suffix}")
                Sp = sps.tile([1, TB], F32, tag="s", name=f"S{suffix}")
                SSp = sps.tile([1, TB], F32, tag="s", name=f"SS{suffix}")
                for mc in range(DC):
                    po = gps.tile([128, TB], F32, tag="g", name=f"po{suffix}")
                    _mmb(nc, po[:], kvw[:, b * D + mc * 128:b * D + mc * 128 + 128],
                         qps_t[:], True, True)
                    nc.vector.tensor_tensor(r1[:, mc * TB:(mc + 1) * TB], po[:],
                                            inp_b[b][:, mc * TB:(mc + 1) * TB],
                                            AluOpType.add)
                    sq = sqp.tile([128, TB], F32R, tag="sq", name=f"sq{suffix}")
                    nc.scalar.activation(sq[:], r1[:, mc * TB:(mc + 1) * TB],
                                         AF.Square)
                    _mm(nc, Sp[:], ones_col[:, 0:1], r1[:, mc * TB:(mc + 1) * TB],
                        mc == 0, mc == DC - 1, True)
                    _mm(nc, SSp[:], ones_col[:, 0:1], sq[:], mc == 0, mc == DC - 1,
                        True)

                mneg = stp.tile([1, TB], F32, tag="s0", name=f"mneg{suffix}")[:]
                m2 = stp.tile([1, TB], F32, tag="s1", name=f"m2{suffix}")[:]
                ve = stp.tile([1, TB], F32, tag="s2", name=f"ve{suffix}")[:]
                sqv = stp.tile([1, TB], F32, tag="s3", name=f"sqv{suffix}")[:]
                n1 = stp.tile([1, TB], F32, tag="s4", name=f"n1{suffix}")[:]
                n2 = stp.tile([1, TB], F32, tag="s5", name=f"n2{suffix}")[:]
                a_ = stp.tile([1, TB], F32R, tag="sta", name=f"a{suffix}")
                bb = stp.tile([1, TB], F32R, tag="stb", name=f"bb{suffix}")
                nc.vector.tensor_scalar(mneg, Sp[:], -1.0 / D, None,
                                        AluOpType.mult)
                nc.vector.tensor_tensor(m2, mneg, mneg, AluOpType.mult)
                nc.vector.scalar_tensor_tensor(ve, in0=SSp[:], scalar=1.0 / D,
                                               in1=m2, op0=AluOpType.mult,
                                               op1=AluOpType.subtract)
                nc.scalar.activation(sqv, ve, AF.Sqrt, bias=eps_t[:])
                nc.vector.reciprocal(a_, sqv)
                nc.vector.tensor_tensor(n1, a_, a_, AluOpType.mult)
                nc.vector.scalar_tensor_tensor(n2, in0=ve, scalar=EPS_LN,
                                               in1=n1, op0=AluOpType.add,
                                               op1=AluOpType.mult)
                nc.vector.tensor_scalar(n2, n2, -0.5, 1.5,
                                        AluOpType.mult, AluOpType.add)
                nc.vector.tensor_tensor(a_, a_, n2, AluOpType.mult)
                nc.vector.tensor_tensor(bb, mneg, a_, AluOpType.mult)
                abc = sps.tile([128, TB], F32, tag="s", name=f"abc{suffix}")
                _mm(nc, abc[:], ones_row[:], a_[:], True, True)
                bbc = sps.tile([128, TB], F32, tag="s", name=f"bbc{suffix}")
                _mm(nc, bbc[:], ones_row[:], bb[:], True, True)

                ob = out_pool.tile([128, DC * TB], out_dt, tag=out_tag,
                                   name=f"o{out_tag}{suffix}{b}")
                out_b_list.append(ob)
                for mc in range(DC):
                    tpm = sqp.tile([128, TB], F32, tag="sq", name=f"tpm{suffix}")
                    nc.vector.tensor_tensor(tpm[:], r1[:, mc * TB:(mc + 1) * TB],
                                            abc[:], AluOpType.mult)
                    nc.vector.tensor_tensor(tpm[:], tpm[:], bbc[:], AluOpType.add)
                    nc.scalar.activation(ob[:, mc * TB:(mc + 1) * TB], tpm[:],
                                         AF.Identity,
                                         bias=beslice(ln_idx)[:, mc:mc + 1],
                                         scale=gslice(ln_idx)[:, mc:mc + 1])
        return out_b_list

    def allreduce(arin, arout):
        nc.gpsimd.collective_compute(
            "AllReduce", AluOpType.add,
            replica_groups=[list(range(NCORES))],
            ins=[arin[:]], outs=[arout[:]])

    # ================= attention 1 + 2 =================
    # o2 pool opens first so it can outlive the mid pools (LIFO release)
    o2p = ExitStack()
    o2pool = o2p.enter_context(tc.tile_pool(name=f"o2p{sfx}", bufs=4))
    mid = ExitStack()
    resid = mid.enter_context(tc.tile_pool(name=f"resid{sfx}", bufs=5))
    qpp = mid.enter_context(tc.tile_pool(name=f"qpp{sfx}", bufs=2))

    wqk1 = ExitStack()
    wqkp = wqk1.enter_context(tc.tile_pool(name=f"wqk{sfx}", bufs=1))
    wqe1 = load_wide(wqkp, tensors['wqe1'], HM, f"wqe1{sfx}")
    wke1 = load_wide(wqkp, tensors['wke1'], HM, f"wke1{sfx}")
    wqe2 = load_wide(wqkp, tensors['wqe2'], HM, f"wqe2{sfx}")

    x_b = []
    for b in range(B):
        xb = resid.tile([128, DC * TB], F32R, tag="resid", name=f"x{sfx}{b}")
        for kc in range(DC):
            nc.sync.dma_start(xb[:, kc * TB:(kc + 1) * TB],
                              tensors['xT'][kc * 128:(kc + 1) * 128,
                                            b * TB:(b + 1) * TB])
        x_b.append(xb)

    qp1 = qpp.tile([HM, T], F32R, tag="qp", name=f"qp1{sfx}")
    feat_phase(x_b, wqe1, wke1, qp1, arin1, f"a1{sfx}")
    allreduce(arin1, arout1)

    # overlap with AR1: qp2 from enc_output (streamed)
    qp2 = qpp.tile([HM, T], F32R, tag="qp", name=f"qp2{sfx}")
    with ExitStack() as ph:
        ep = ph.enter_context(tc.tile_pool(name=f"encp{sfx}", bufs=2))
        pqs = ph.enter_context(tc.tile_pool(name=f"pq2{sfx}", bufs=2, space="PSUM"))
        for b in range(B):
            eb = ep.tile([128, DC * TB], F32R, tag="enc", name=f"enc{sfx}")
            for kc in range(DC):
                nc.sync.dma_start(eb[:, kc * TB:(kc + 1) * TB],
                                  tensors['encT'][kc * 128:(kc + 1) * 128,
                                                  b * TB:(b + 1) * TB])
            pq = pqs.tile([128, TB], F32, tag="pq2", name=f"pq2{sfx}")
            for kc in range(DC):
                _mm(nc, pq[:], wqe2[:, kc * HM:(kc + 1) * HM],
                    eb[:, kc * TB:(kc + 1) * TB], kc == 0, kc == DC - 1)
            nc.vector.tensor_scalar(qp2[:, b * TB:(b + 1) * TB], pq[:],
                                    0.0, STAB, AluOpType.max, AluOpType.add)
    wqk1.close()

    out1_b = attn_out_phase(tensors['wv1'], tensors['wo1'], arout1, qp1, x_b,
                            resid, "resid", F32R, 0, f"a1{sfx}")

    wqk2 = ExitStack()
    wqkp2 = wqk2.enter_context(tc.tile_pool(name=f"wk2p{sfx}", bufs=1))
    wke2 = load_wide(wqkp2, tensors['wke2'], HM, f"wke2{sfx}")
    feat_phase(out1_b, None, wke2, None, arin2, f"a2{sfx}")
    allreduce(arin2, arout2)
    wqk2.close()

    out2_b = attn_out_phase(tensors['wv2'], tensors['wo2'], arout2, qp2,
                            out1_b, o2pool, "o2", BF16, 1, f"a2{sfx}")
    mid.close()

    # ================= FFN (h = elu(out2 @ W1 + b1), spilled bf16) ==========
    SL = 256
    NSL = T // SL
    with ExitStack() as ph:
        wp = ph.enter_context(tc.tile_pool(name=f"w1p{sfx}", bufs=1))
        hstg = ph.enter_context(tc.tile_pool(name=f"hstg{sfx}", bufs=3))
        ep_ = ph.enter_context(tc.tile_pool(name=f"ep{sfx}", bufs=3))
        hps = ph.enter_context(tc.tile_pool(name=f"hps{sfx}", bufs=4, space="PSUM"))
        w1_sb = load_wide(wp, tensors['w1'], DFF, f"w1{sfx}", BF16)
        for s in range(NSL):
            b = s // (TB // SL)
            off = (s % (TB // SL)) * SL
            o2b = out2_b[b]
            for dffc in range(DFC):
                hps_t = hps.tile([128, SL], F32, tag="h", name=f"hps{sfx}")
                for kc in range(DC):
                    _mmb(nc, hps_t[:],
                         w1_sb[:, kc * DFF + dffc * 128:kc * DFF + dffc * 128 + 128],
                         o2b[:, kc * TB + off:kc * TB + off + SL],
                         kc == 0, kc == DC - 1)
                # ELU(u + b1) = min(exp(u+b1) - 1, max(u+b1, 0))
                e_ = ep_.tile([128, SL], F32, tag="e", name=f"e{sfx}")
                nc.scalar.activation(e_[:], hps_t[:], AF.Exp,
                                     bias=b1c[:, dffc:dffc + 1])
                t_ = ep_.tile([128, SL], F32, tag="t", name=f"t{sfx}")
                nc.vector.tensor_scalar(t_[:], hps_t[:], b1c[:, dffc:dffc + 1],
                                        0.0, AluOpType.add, AluOpType.max)
                h_ = hstg.tile([128, SL], BF16, tag="hsb", name=f"h{sfx}")
                nc.vector.scalar_tensor_tensor(h_[:], in0=e_[:], scalar=1.0,
                                               in1=t_[:], op0=AluOpType.subtract,
                                               op1=AluOpType.min)
                nc.sync.dma_start(
                    h_spill[dffc * 128:(dffc + 1) * 128,
                            s * SL:(s + 1) * SL], h_[:])

    # ============ r3 = h @ W2 + b2 + out2 ; token-major LN3 -> out ==========
    with ExitStack() as ph:
        wp = ph.enter_context(tc.tile_pool(name=f"w2p{sfx}", bufs=1))
        hin = ph.enter_context(tc.tile_pool(name=f"hin{sfx}", bufs=2))
        r3p = ph.enter_context(tc.tile_pool(name=f"r3p{sfx}", bufs=2))
        o3p = ph.enter_context(tc.tile_pool(name=f"o3p{sfx}", bufs=2))
        sqp = ph.enter_context(tc.tile_pool(name=f"sq3{sfx}", bufs=2))
        stp = ph.enter_context(tc.tile_pool(name=f"st3{sfx}", bufs=8))
        rps = ph.enter_context(tc.tile_pool(name=f"rps{sfx}", bufs=2, space="PSUM"))
        ops = ph.enter_context(tc.tile_pool(name=f"ops{sfx}", bufs=2, space="PSUM"))
        w2_sb = load_wide(wp, tensors['w2'], D, f"w2{sfx}", BF16)

        for s in range(NSL):
            b = s // (TB // SL)
            off = (s % (TB // SL)) * SL
            o2b = out2_b[b]
            h_sb = hin.tile([128, DFC * SL], BF16, tag="hin", name=f"hin{sfx}")
            for dffc in range(DFC):
                nc.sync.dma_start(h_sb[:, dffc * SL:(dffc + 1) * SL],
                                  h_spill[dffc * 128:(dffc + 1) * 128,
                                          s * SL:(s + 1) * SL])
            for t3 in range(SL // 128):
                toff = off + t3 * 128
                tok0 = b * TB + toff
                rt = [rps.tile([128, 512], F32, tag="r3", name=f"r3{sfx}{half}")
                      for half in range(2)]
                for half in range(2):
                    for dffc in range(DFC):
                        _mmb(nc, rt[half][:],
                             h_sb[:, dffc * SL + t3 * 128:dffc * SL + t3 * 128 + 128],
                             w2_sb[:, dffc * D + half * 512:dffc * D + half * 512 + 512],
                             dffc == 0, False, True)
                    _mm(nc, rt[half][:], ones_row[:],
                        b2r[:, half * 512:half * 512 + 512], False, True, True)
                # transpose out2 block (bf16) for the token-major residual
                o2t = ops.tile([128, D], BF16, tag="o2t", name=f"o2t{sfx}")
                for kc in range(DC):
                    nc.tensor.matmul(o2t[:, kc * 128:(kc + 1) * 128],
                                     o2b[:, kc * TB + toff:kc * TB + toff + 128],
                                     identb[:], start=True, stop=True,
                                     is_transpose=True, skip_group_check=True)
                o2ts = o3p.tile([128, D], BF16, tag="o2ts", name=f"o2ts{sfx}")
                nc.any.tensor_copy(o2ts[:], o2t[:])
                r3 = r3p.tile([128, D], F32, tag="r3s", name=f"r3s{sfx}")
                for half in range(2):
                    nc.vector.tensor_tensor(r3[:, half * 512:(half + 1) * 512],
                                            rt[half][:],
                                            o2ts[:, half * 512:(half + 1) * 512],
                                            AluOpType.add)
                # token-major LN3: stats along the free (feature) dim
                Sc = stp.tile([128, 1], F32, tag="st3", name=f"Sc{sfx}")
                nc.vector.tensor_reduce(Sc[:], r3[:], AX.X, AluOpType.add)
                mneg = stp.tile([128, 1], F32, tag="st3", name=f"mneg3{sfx}")
                nc.vector.tensor_scalar(mneg[:], Sc[:], -1.0 / D, None,
                                        AluOpType.mult)
                sq = sqp.tile([128, D], F32R, tag="sq3", name=f"sq3{sfx}")
                nc.scalar.activation(sq[:], r3[:], AF.Square)
                SSc = stp.tile([128, 1], F32, tag="st3", name=f"SSc{sfx}")
                nc.vector.tensor_reduce(SSc[:], sq[:], AX.X, AluOpType.add)
                m2 = stp.tile([128, 1], F32, tag="st3", name=f"m23{sfx}")
                nc.vector.tensor_tensor(m2[:], mneg[:], mneg[:], AluOpType.mult)
                ve = stp.tile([128, 1], F32, tag="st3", name=f"ve3{sfx}")
                nc.vector.scalar_tensor_tensor(ve[:], in0=SSc[:], scalar=1.0 / D,
                                               in1=m2[:], op0=AluOpType.mult,
                                               op1=AluOpType.subtract)
                sqv = stp.tile([128, 1], F32, tag="st3", name=f"sqv3{sfx}")
                nc.scalar.activation(sqv[:], ve[:], AF.Sqrt, bias=eps_c[:])
                a_ = stp.tile([128, 1], F32, tag="st3", name=f"a3{sfx}")
                nc.vector.reciprocal(a_[:], sqv[:])
                n1 = stp.tile([128, 1], F32, tag="st3", name=f"n13{sfx}")
                nc.vector.tensor_tensor(n1[:], a_[:], a_[:], AluOpType.mult)
                n2 = stp.tile([128, 1], F32, tag="st3", name=f"n23{sfx}")
                nc.vector.scalar_tensor_tensor(n2[:], in0=ve[:], scalar=EPS_LN,
                                               in1=n1[:], op0=AluOpType.add,
                                               op1=AluOpType.mult)
                nc.vector.tensor_scalar(n2[:], n2[:], -0.5, 1.5,
                                        AluOpType.mult, AluOpType.add)
                nc.vector.tensor_tensor(a_[:], a_[:], n2[:], AluOpType.mult)
                # apply: out = ((r3 - m) * rstd) * g3 + be3
                o3 = o3p.tile([128, D], F32, tag="o3", name=f"o3{sfx}")
                nc.vector.tensor_scalar(o3[:], r3[:], mneg[:], a_[:],
                                        AluOpType.add, AluOpType.mult)
                nc.vector.tensor_tensor(o3[:], o3[:], g3bc[:], AluOpType.mult)
                nc.vector.tensor_tensor(o3[:], o3[:], be3bc[:], AluOpType.add)
                nc.sync.dma_start(out_d[tok0:tok0 + 128, :], o3[:])
    o2p.close()


def _host_prep(inputs):
    """Build per-core in_maps from full inputs."""
    f32 = np.float32
    bf16 = ml_dtypes.bfloat16
    x = np.asarray(inputs['x'], f32)
    enc = np.asarray(inputs['enc_output'], f32)

    def fold(Wq, P):
        # [D, H, DH] x [M, DH] -> [D, H*M]
        w = np.einsum('dhk,mk->dhm', np.asarray(Wq, f32), np.asarray(P, f32))
        return np.ascontiguousarray(w.reshape(D, HM) / np.sqrt(M))

    e16T = np.zeros((HM, H), f32)
    e16 = np.zeros((H, HM), f32)
    for h in range(H):
        e16T[h * M:(h + 1) * M, h] = 1.0
        e16[h, h * M:(h + 1) * M] = 1.0

    khead = np.arange(D) // DH         # head of each v/k feature row
    fhead = np.arange(HM) // M         # head of each random feature
    kvm4 = (khead[:, None] == fhead[None, :]).astype(bf16)

    gbe = np.zeros((128, 4 * DC), f32)
    for i, nm in enumerate(['g1', 'be1', 'g2', 'be2']):
        gbe[:, i * DC:(i + 1) * DC] = np.asarray(inputs[nm], f32).reshape(DC, 128).T

    shared = {
        'wqe1': fold(inputs['Wq1'], inputs['P1']),
        'wke1': fold(inputs['Wk1'], inputs['P1']),
        'wqe2': fold(inputs['Wq2'], inputs['P2']),
        'wke2': fold(inputs['Wk2'], inputs['P2']),
        'wv1': np.asarray(inputs['Wv1'], f32).reshape(D, D).astype(bf16),
        'wo1': np.asarray(inputs['Wo1'], f32).reshape(D, D).astype(bf16),
        'wv2': np.asarray(inputs['Wv2'], f32).reshape(D, D).astype(bf16),
        'wo2': np.asarray(inputs['Wo2'], f32).reshape(D, D).astype(bf16),
        'w1': np.asarray(inputs['W1'], f32).astype(bf16),
        'w2': np.asarray(inputs['W2'], f32).astype(bf16),
        'kvm4': kvm4,
        'e16T': e16T, 'e16': e16,
        'gbe': gbe,
        'b1c': np.ascontiguousarray(np.asarray(inputs['b1'], f32).reshape(DFC, 128).T),
        'b2r': np.asarray(inputs['b2'], f32).reshape(1, D),
        'g3r': np.asarray(inputs['g3'], f32).reshape(1, D),
        'be3r': np.asarray(inputs['be3'], f32).reshape(1, D),
        'ident': np.eye(128, dtype=f32),
        'identb': np.eye(128, dtype=bf16),
        'ones_col': np.ones((128, 8), f32),
        'ones_row': np.ones((1, 128), f32),
    }

    in_maps = []
    for i in range(NCORES):
        sl = slice(i * LSH, (i + 1) * LSH)
        m = dict(shared)
        m['xT'] = np.ascontiguousarray(
            x[:, sl, :].transpose(2, 0, 1).reshape(D, T))
        m['encT'] = np.ascontiguousarray(
            enc[:, sl, :].transpose(2, 0, 1).reshape(D, T))
        in_maps.append(m)
    return in_maps


def kernel(**inputs) -> np.ndarray:
    if 'nc' not in _cache:
        _cache['nc'] = build_program()
    nc = _cache['nc']
    in_maps = _host_prep(inputs)
    res = run_bass_kernel_spmd(nc, in_maps, core_ids=list(range(NCORES)))
    out = np.empty((B, L, D), np.float32)
    for i in range(NCORES):
        o = res.results[i]['out']  # [T, D] token-major
        out[:, i * LSH:(i + 1) * LSH, :] = o.reshape(B, LSH, D)
    return out


if __name__ == '__main__':
    print("building program...")
    build_program()
    print("OK")


# revision 3
# speedup vs baseline: 2427.2967x; 1.5324x over previous
"""Trainium2 Bass kernel for nn_DecoderLayer (Performer/FAVOR+ decoder layer).

v2: folded-projection FAVOR+. Because the Performer uses only M=8 random
features per head (HM = H*M = 128 total), the Q/K projections fold into the
random-feature projection on the host: wqe = Wq @ P^T/sqrt(M) is [D, 128], so
qp = relu(x @ wqe) + stab needs no D x D matmul. The V and O projections fold
through the kv statistic: kv = kp^T V = (kp^T X) Wv and the output
o @ Wo = qps @ ((kv blockdiag-masked) @ Wo), so the only D x D work left is
applied to the tiny [D x 128] ST statistic instead of the full sequence.

Sharding: sequence (L) split across 8 cores; the per-batch global statistics
ST = [X^T kp | kp-sums] (~2MB total for B=4) are AllReduced twice.
Residual stream is feature-major; the final LN3 runs token-major so the
output DMAs out in natural [T, D] layout (no host post-transpose).
"""
import sys
import os

sys.path.insert(0, '/opt/trn_rl_repo')

import numpy as np
import ml_dtypes
from contextlib import ExitStack

from concourse import bass, bacc, tile
import concourse.mybir as mybir
from concourse.bass_utils import run_bass_kernel_spmd
from concourse.alu_op_type import AluOpType

F32 = mybir.dt.float32
F32R = mybir.dt.float32r
BF16 = mybir.dt.bfloat16
AF = mybir.ActivationFunctionType
AX = mybir.AxisListType

B, L, D, H, DH, M, DFF = 4, 4096, 1024, 16, 64, 8, 4096
NCORES = 8
LSH = L // NCORES          # 512 tokens of L per core
T = B * LSH                # 2048 tokens per core
TB = LSH                   # tokens per batch element per core (512)
NTS = TB // 128            # 4 x 128-token blocks per batch
DC = D // 128              # 8 d-chunks
DFC = DFF // 128           # 32 dff-chunks
HM = H * M                 # 128 random features total
EPS_LN = 1e-6
STAB = 0.001
ARW = DC * 512 + 4         # allreduce width: ST (8 chunks x 4 batches x 128) + ksums

_cache = {}


def _mm(nc, out, lhsT, rhs, start, stop, skip=False):
    nc.tensor.matmul(out, lhsT.bitcast(F32R), rhs.bitcast(F32R),
                     start=start, stop=stop, skip_group_check=skip)


def _mmb(nc, out, lhsT, rhs, start, stop, skip=False):
    nc.tensor.matmul(out, lhsT, rhs, start=start, stop=stop,
                     skip_group_check=skip)


def build_program(loop=1):
    nc = bacc.Bacc("TRN2", target_bir_lowering=False, debug=False,
                   num_devices=NCORES)

    def din(name, shape, dt=F32R):
        return nc.dram_tensor(name, shape, dt, kind="ExternalInput").ap()

    tensors = dict(
        xT=din("xT", [D, T]),
        encT=din("encT", [D, T]),
        wqe1=din("wqe1", [D, HM]), wke1=din("wke1", [D, HM]),
        wqe2=din("wqe2", [D, HM]), wke2=din("wke2", [D, HM]),
        wv1=din("wv1", [D, D], BF16), wo1=din("wo1", [D, D], BF16),
        wv2=din("wv2", [D, D], BF16), wo2=din("wo2", [D, D], BF16),
        w1=din("w1", [D, DFF], BF16), w2=din("w2", [DFF, D], BF16),
        kvm4=din("kvm4", [D, HM], BF16),
        e16T_d=din("e16T", [HM, H]), e16_d=din("e16", [H, HM]),
        gbe_d=din("gbe", [128, 4 * DC], F32),
        b1c_d=din("b1c", [128, DFC], F32),
        b2r_d=din("b2r", [1, D]),
        g3r_d=din("g3r", [1, D]), be3r_d=din("be3r", [1, D]),
        ident_d=din("ident", [128, 128]),
        identb_d=din("identb", [128, 128], BF16),
        ones_col_d=din("ones_col", [128, 8]),
        ones_row_d=din("ones_row", [1, 128]),
    )

    out_d = nc.dram_tensor("out", [T, D], F32, kind="ExternalOutput").ap()

    with nc.allow_low_precision(reason="f32r/bf16 matmul inputs"), \
         tile.TileContext(nc) as tc, ExitStack() as top:
        dram = top.enter_context(tc.tile_pool(name="dram", bufs=1, space="DRAM"))

        const = top.enter_context(tc.tile_pool(name="const", bufs=1))
        c = {}
        c['e16T'] = const.tile([HM, H], F32R, name="e16T")
        nc.sync.dma_start(c['e16T'][:], tensors['e16T_d'][:])
        c['e16'] = const.tile([H, HM], F32R, name="e16")
        nc.sync.dma_start(c['e16'][:], tensors['e16_d'][:])
        c['gbe'] = const.tile([128, 4 * DC], F32, name="gbe")
        nc.sync.dma_start(c['gbe'][:], tensors['gbe_d'][:])
        c['b1c'] = const.tile([128, DFC], F32, name="b1c")
        nc.sync.dma_start(c['b1c'][:], tensors['b1c_d'][:])
        c['b2r'] = const.tile([1, D], F32R, name="b2r")
        nc.sync.dma_start(c['b2r'][:], tensors['b2r_d'][:])
        c['ident'] = const.tile([128, 128], F32R, name="ident")
        nc.sync.dma_start(c['ident'][:], tensors['ident_d'][:])
        c['identb'] = const.tile([128, 128], BF16, name="identb")
        nc.sync.dma_start(c['identb'][:], tensors['identb_d'][:])
        c['ones_col'] = const.tile([128, 8], F32R, name="ones_col")
        nc.sync.dma_start(c['ones_col'][:], tensors['ones_col_d'][:])
        c['ones_row'] = const.tile([1, 128], F32R, name="ones_row")
        nc.sync.dma_start(c['ones_row'][:], tensors['ones_row_d'][:])
        c['eps_t'] = const.tile([1, 1], F32, name="eps_t")
        nc.vector.memset(c['eps_t'][:], EPS_LN)
        c['eps_c'] = const.tile([128, 1], F32, name="eps_c")
        nc.vector.memset(c['eps_c'][:], EPS_LN)
        g3r = const.tile([1, D], F32R, name="g3r")
        nc.sync.dma_start(g3r[:], tensors['g3r_d'][:])
        be3r = const.tile([1, D], F32R, name="be3r")
        nc.sync.dma_start(be3r[:], tensors['be3r_d'][:])
        c['g3bc'] = const.tile([128, D], F32R, name="g3bc")
        c['be3bc'] = const.tile([128, D], F32R, name="be3bc")
        with tc.tile_pool(name="bc_ps", bufs=2, space="PSUM") as bcp:
            for src, dst in ((g3r, c['g3bc']), (be3r, c['be3bc'])):
                for half in range(2):
                    pg = bcp.tile([128, D // 2], F32, tag="bc", name="pbc")
                    _mm(nc, pg[:], c['ones_row'][:],
                        src[:, half * 512:(half + 1) * 512], True, True)
                    nc.any.tensor_copy(dst[:, half * 512:(half + 1) * 512], pg[:])

        for it in range(loop):
            build_iter(nc, tc, f"i{it}" if loop > 1 else "", tensors, c,
                       dram, out_d)

    nc.compile()
    return nc


def build_iter(nc, tc, sfx, tensors, c, dram, out_d):
    e16T, e16 = c['e16T'], c['e16']
    gbe, b1c, b2r = c['gbe'], c['b1c'], c['b2r']
    ident, identb = c['ident'], c['identb']
    ones_col, ones_row = c['ones_col'], c['ones_row']
    eps_t, g3bc, be3bc = c['eps_t'], c['g3bc'], c['be3bc']
    eps_c = c['eps_c']

    arin1 = dram.tile([128, ARW], F32, name=f"arin1{sfx}")
    arout1 = dram.tile([128, ARW], F32, addr_space="Shared", name=f"arout1{sfx}")
    arin2 = dram.tile([128, ARW], F32, name=f"arin2{sfx}")
    arout2 = dram.tile([128, ARW], F32, addr_space="Shared", name=f"arout2{sfx}")
    h_spill = dram.tile([DFF, T], BF16, name=f"hspill{sfx}")

    def gslice(i):
        return gbe[:, 2 * i * DC:(2 * i + 1) * DC]

    def beslice(i):
        return gbe[:, (2 * i + 1) * DC:(2 * i + 2) * DC]

    def load_wide(pool, src_dram, ncols, name, dt=F32R):
        nchunk = src_dram.shape[0] // 128
        t_ = pool.tile([128, nchunk * ncols], dt, name=name)
        for kc in range(nchunk):
            nc.sync.dma_start(t_[:, kc * ncols:(kc + 1) * ncols],
                              src_dram[kc * 128:(kc + 1) * 128, :])
        return t_

    def feat_phase(inp_b, wqe, wke, qp_fm, arin, suffix):
        """qp/kp features + ST statistic + ksum -> arin (DRAM)."""
        with ExitStack() as ph:
            work = ph.enter_context(tc.tile_pool(name=f"ftw{suffix}", bufs=2))
            kptp = ph.enter_context(tc.tile_pool(name=f"kptp{suffix}", bufs=2))
            arp = ph.enter_context(tc.tile_pool(name=f"arp{suffix}", bufs=1))
            pqk = ph.enter_context(tc.tile_pool(name=f"pqk{suffix}", bufs=2, space="PSUM"))
            tpp = ph.enter_context(tc.tile_pool(name=f"tpp{suffix}", bufs=4, space="PSUM"))
            stps = ph.enter_context(tc.tile_pool(name=f"stps{suffix}", bufs=2, space="PSUM"))

            arin_sb = arp.tile([128, ARW], F32, name=f"arin_sb{suffix}")
            for b in range(B):
                xb = inp_b[b]
                if qp_fm is not None:
                    pq = pqk.tile([128, TB], F32, tag="pqk", name=f"pq{suffix}")
                    for kc in range(DC):
                        _mm(nc, pq[:], wqe[:, kc * HM:(kc + 1) * HM],
                            xb[:, kc * TB:(kc + 1) * TB], kc == 0, kc == DC - 1)
                    nc.vector.tensor_scalar(qp_fm[:, b * TB:(b + 1) * TB], pq[:],
                                            0.0, STAB, AluOpType.max, AluOpType.add)
                pk = pqk.tile([128, TB], F32, tag="pqk", name=f"pk{suffix}")
                for kc in range(DC):
                    _mm(nc, pk[:], wke[:, kc * HM:(kc + 1) * HM],
                        xb[:, kc * TB:(kc + 1) * TB], kc == 0, kc == DC - 1)
                kpf = work.tile([128, TB], F32R, tag="kpf", name=f"kpf{suffix}")
                nc.vector.tensor_scalar(kpf[:], pk[:], 0.0, STAB,
                                        AluOpType.max, AluOpType.add)
                # global k-feature sum for z (per-batch column of arin)
                nc.vector.tensor_reduce(arin_sb[:, DC * 512 + b:DC * 512 + b + 1],
                                        kpf[:], AX.X, AluOpType.add)
                # kp token-major [128 tok, HM] per 128-token block (bf16: the
                # ST statistic matmuls run 1 cyc/row at 128-col ap in bf16)
                kptm = kptp.tile([128, NTS * 128], BF16, tag="kptm",
                                 name=f"kptm{suffix}")
                for ts in range(NTS):
                    ptt = tpp.tile([128, 128], F32R, tag="tp", name=f"ptt{suffix}")
                    nc.tensor.transpose(ptt[:], kpf[:, ts * 128:(ts + 1) * 128],
                                        ident[:])
                    nc.any.tensor_copy(kptm[:, ts * 128:(ts + 1) * 128], ptt[:])
                # ST[d, hm] += x[l, d]*kp[l, hm]: transpose x chunks, accumulate
                stA = stps.tile([128, 512], F32, tag="st", name=f"stA{suffix}")
                stB = stps.tile([128, 512], F32, tag="st", name=f"stB{suffix}")
                for kc in range(DC):
                    stt = stA if kc < 4 else stB
                    col = (kc % 4) * 128
                    for ts in range(NTS):
                        xtt = tpp.tile([128, 128], F32R, tag="tp",
                                       name=f"xtt{suffix}")
                        nc.tensor.transpose(
                            xtt[:], xb[:, kc * TB + ts * 128:kc * TB + ts * 128 + 128],
                            ident[:])
                        xts = work.tile([128, 128], BF16, tag="xts",
                                        name=f"xts{suffix}")
                        nc.any.tensor_copy(xts[:], xtt[:])
                        _mmb(nc, stt[:, col:col + 128], xts[:],
                             kptm[:, ts * 128:(ts + 1) * 128],
                             ts == 0, ts == NTS - 1, True)
                for kc in range(DC):
                    stt = stA if kc < 4 else stB
                    col = (kc % 4) * 128
                    nc.any.tensor_copy(
                        arin_sb[:, kc * 512 + b * 128:kc * 512 + b * 128 + 128],
                        stt[:, col:col + 128])
            nc.sync.dma_start(arin[:], arin_sb[:])

    def attn_out_phase(wv_d, wo_d, arout, qp_fm, inp_b, out_pool, out_tag,
                       out_dt, ln_idx, suffix):
        """kv reconstruction + o = qps @ kvW + residual + LN -> out tiles."""
        out_b_list = []
        with ExitStack() as ph:
            gps = ph.enter_context(tc.tile_pool(name=f"gps{suffix}", bufs=3, space="PSUM"))
            sps = ph.enter_context(tc.tile_pool(name=f"sps{suffix}", bufs=5, space="PSUM"))
            ksp = ph.enter_context(tc.tile_pool(name=f"ksp{suffix}", bufs=1))
            kvwp = ph.enter_context(tc.tile_pool(name=f"kvwp{suffix}", bufs=1))

            ksums = ksp.tile([128, 4], F32, name=f"ksums{suffix}")
            kvw = kvwp.tile([128, B * D], BF16, name=f"kvw{suffix}")
            with ExitStack() as s1:
                kvbp = s1.enter_context(tc.tile_pool(name=f"kvbp{suffix}", bufs=1))
                kvbd = kvbp.tile([128, DC * 512], BF16, name=f"kvbd{suffix}")
                with ExitStack() as s0:
                    stbp = s0.enter_context(tc.tile_pool(name=f"stbp{suffix}", bufs=1))
                    st_b = stbp.tile([128, DC * 512], BF16, name=f"stb{suffix}")
                    # load ST (f32 staging freed before weights arrive)
                    with ExitStack() as sg:
                        stg = sg.enter_context(tc.tile_pool(name=f"stg{suffix}", bufs=1))
                        st_f = stg.tile([128, ARW], F32, name=f"stf{suffix}")
                        nc.sync.dma_start(st_f[:], arout[:])
                        nc.any.tensor_copy(ksums[:],
                                           st_f[:, DC * 512:DC * 512 + 4])
                        nc.any.tensor_copy(st_b[:], st_f[:, 0:DC * 512])
                    # stage 1: kvT (masked per batch) with Wv resident
                    with ExitStack() as sv:
                        wvp = sv.enter_context(tc.tile_pool(name=f"wvp{suffix}", bufs=1))
                        wv_sb = load_wide(wvp, wv_d, D, f"wv{suffix}", BF16)
                        kvm_sb = load_wide(wvp, tensors['kvm4'], HM,
                                           f"kvm{suffix}", BF16)
                        for kc in range(DC):
                            pkv = gps.tile([128, 512], F32, tag="g",
                                           name=f"pkv{suffix}")
                            for dc in range(DC):
                                _mmb(nc, pkv[:],
                                     wv_sb[:, dc * D + kc * 128:dc * D + kc * 128 + 128],
                                     st_b[:, dc * 512:(dc + 1) * 512],
                                     dc == 0, dc == DC - 1)
                            for b in range(B):
                                nc.vector.tensor_tensor(
                                    kvbd[:, kc * 512 + b * 128:kc * 512 + b * 128 + 128],
                                    pkv[:, b * 128:(b + 1) * 128],
                                    kvm_sb[:, kc * HM:(kc + 1) * HM],
                                    AluOpType.mult)
                # stage 2: kvW = kvbd^T @ Wo per batch, with Wo resident
                with ExitStack() as s2:
                    wop = s2.enter_context(tc.tile_pool(name=f"wop{suffix}", bufs=1))
                    wo_sb = load_wide(wop, wo_d, D, f"wo{suffix}", BF16)
                    for b in range(B):
                        for half in range(2):
                            pw = gps.tile([128, 512], F32, tag="g",
                                          name=f"pw{suffix}")
                            for kc in range(DC):
                                _mmb(nc, pw[:],
                                     kvbd[:, kc * 512 + b * 128:kc * 512 + b * 128 + 128],
                                     wo_sb[:, kc * D + half * 512:kc * D + half * 512 + 512],
                                     kc == 0, kc == DC - 1)
                            nc.any.tensor_copy(
                                kvw[:, b * D + half * 512:b * D + half * 512 + 512],
                                pw[:])

            # stage 3: z, o, residual, LN per batch
            fv = ph.enter_context(tc.tile_pool(name=f"fv{suffix}", bufs=1))
            r1p = ph.enter_context(tc.tile_pool(name=f"r1{suffix}", bufs=1))
            sqp = ph.enter_context(tc.tile_pool(name=f"sq{suffix}", bufs=2))
            stp = ph.enter_context(tc.tile_pool(name=f"stt{suffix}", bufs=1))
            for b in range(B):
                qpk = fv.tile([128, TB], F32R, tag="qpk", name=f"qpk{suffix}")
                nc.vector.tensor_scalar(qpk[:], qp_fm[:, b * TB:(b + 1) * TB],
                                        ksums[:, b:b + 1], None, AluOpType.mult)
                zps = sps.tile([H, TB], F32, tag="s", name=f"z{suffix}")
                _mm(nc, zps[:], e16T[:], qpk[:], True, True)
                rz = fv.tile([H, TB], F32R, tag="rz", name=f"rz{suffix}")
                nc.vector.reciprocal(rz[:], zps[:])
                t1 = fv.tile([H, TB], F32, tag="nt1", name=f"nt1{suffix}")
                nc.vector.tensor_tensor(t1[:], zps[:], rz[:], AluOpType.mult)
                nc.vector.tensor_scalar(t1[:], t1[:], -1.0, 2.0,
                                        AluOpType.mult, AluOpType.add)
                nc.vector.tensor_tensor(rz[:], rz[:], t1[:], AluOpType.mult)
                zbc = sps.tile([128, TB], F32, tag="s", name=f"zbc{suffix}")
                _mm(nc, zbc[:], e16[:], rz[:], True, True)
                qps_t = fv.tile([128, TB], BF16, tag="qps", name=f"qps{suffix}")
                nc.vector.tensor_tensor(qps_t[:], qp_fm[:, b * TB:(b + 1) * TB],
                                        zbc[:], AluOpType.mult)

                r1 = r1p.tile([128, DC * TB], F32R, tag="r1", name=f"r1{suffix}")
                Sp = sps.tile([1, TB], F32, tag="s", name=f"S{suffix}")
                SSp = sps.tile([1, TB], F32, tag="s", name=f"SS{suffix}")
                for mc in range(DC):
                    po = gps.tile([128, TB], F32, tag="g", name=f"po{suffix}")
                    _mmb(nc, po[:], kvw[:, b * D + mc * 128:b * D + mc * 128 + 128],
                         qps_t[:], True, True)
                    nc.vector.tensor_tensor(r1[:, mc * TB:(mc + 1) * TB], po[:],
                                            inp_b[b][:, mc * TB:(mc + 1) * TB],
                                            AluOpType.add)
                    sq = sqp.tile([128, TB], F32R, tag="sq", name=f"sq{suffix}")
                    nc.scalar.activation(sq[:], r1[:, mc * TB:(mc + 1) * TB],
                                         AF.Square)
                    _mm(nc, Sp[:], ones_col[:, 0:1], r1[:, mc * TB:(mc + 1) * TB],
                        mc == 0, mc == DC - 1, True)
                    _mm(nc, SSp[:], ones_col[:, 0:1], sq[:], mc == 0, mc == DC - 1,
                        True)

                mneg = stp.tile([1, TB], F32, tag="s0", name=f"mneg{suffix}")[:]
                m2 = stp.tile([1, TB], F32, tag="s1", name=f"m2{suffix}")[:]
                ve = stp.tile([1, TB], F32, tag="s2", name=f"ve{suffix}")[:]
                sqv = stp.tile([1, TB], F32, tag="s3", name=f"sqv{suffix}")[:]
                n1 = stp.tile([1, TB], F32, tag="s4", name=f"n1{suffix}")[:]
                n2 = stp.tile([1, TB], F32, tag="s5", name=f"n2{suffix}")[:]
                a_ = stp.tile([1, TB], F32R, tag="sta", name=f"a{suffix}")
                bb = stp.tile([1, TB], F32R, tag="stb", name=f"bb{suffix}")
                nc.vector.tensor_scalar(mneg, Sp[:], -1.0 / D, None,
                                        AluOpType.mult)
                nc.vector.tensor_tensor(m2, mneg, mneg, AluOpType.mult)
                nc.vector.scalar_tensor_tensor(ve, in0=SSp[:], scalar=1.0 / D,
                                               in1=m2, op0=AluOpType.mult,
                                               op1=AluOpType.subtract)
                nc.scalar.activation(sqv, ve, AF.Sqrt, bias=eps_t[:])
                nc.vector.reciprocal(a_, sqv)
                nc.vector.tensor_tensor(n1, a_, a_, AluOpType.mult)
                nc.vector.scalar_tensor_tensor(n2, in0=ve, scalar=EPS_LN,
                                               in1=n1, op0=AluOpType.add,
                                               op1=AluOpType.mult)
                nc.vector.tensor_scalar(n2, n2, -0.5, 1.5,
                                        AluOpType.mult, AluOpType.add)
                nc.vector.tensor_tensor(a_, a_, n2, AluOpType.mult)
                nc.vector.tensor_tensor(bb, mneg, a_, AluOpType.mult)
                abc = sps.tile([128, TB], F32, tag="s", name=f"abc{suffix}")
                _mm(nc, abc[:], ones_row[:], a_[:], True, True)
                bbc = sps.tile([128, TB], F32, tag="s", name=f"bbc{suffix}")
                _mm(nc, bbc[:], ones_row[:], bb[:], True, True)

                ob = out_pool.tile([128, DC * TB], out_dt, tag=out_tag,
                                   name=f"o{out_tag}{suffix}{b}")
                out_b_list.append(ob)
                for mc in range(DC):
                    tpm = sqp.tile([128, TB], F32, tag="sq", name=f"tpm{suffix}")
                    nc.vector.tensor_tensor(tpm[:], r1[:, mc * TB:(mc + 1) * TB],
                                            abc[:], AluOpType.mult)
                    nc.vector.tensor_tensor(tpm[:], tpm[:], bbc[:], AluOpType.add)
                    nc.scalar.activation(ob[:, mc * TB:(mc + 1) * TB], tpm[:],
                                         AF.Identity,
                                         bias=beslice(ln_idx)[:, mc:mc + 1],
                                         scale=gslice(ln_idx)[:, mc:mc + 1])
        return out_b_list

    def allreduce(arin, arout):
        nc.gpsimd.collective_compute(
            "AllReduce", AluOpType.add,
            replica_groups=[list(range(NCORES))],
            ins=[arin[:]], outs=[arout[:]])

    # ================= attention 1 + 2 =================
    # o2 pool opens first so it can outlive the mid pools (LIFO release)
    o2p = ExitStack()
    o2pool = o2p.enter_context(tc.tile_pool(name=f"o2p{sfx}", bufs=4))
    mid = ExitStack()
    resid = mid.enter_context(tc.tile_pool(name=f"resid{sfx}", bufs=5))
    qpp = mid.enter_context(tc.tile_pool(name=f"qpp{sfx}", bufs=2))

    wqk1 = ExitStack()
    wqkp = wqk1.enter_context(tc.tile_pool(name=f"wqk{sfx}", bufs=1))
    wqe1 = load_wide(wqkp, tensors['wqe1'], HM, f"wqe1{sfx}")
    wke1 = load_wide(wqkp, tensors['wke1'], HM, f"wke1{sfx}")
    wqe2 = load_wide(wqkp, tensors['wqe2'], HM, f"wqe2{sfx}")

    x_b = []
    for b in range(B):
        xb = resid.tile([128, DC * TB], F32R, tag="resid", name=f"x{sfx}{b}")
        for kc in range(DC):
            nc.sync.dma_start(xb[:, kc * TB:(kc + 1) * TB],
                              tensors['xT'][kc * 128:(kc + 1) * 128,
                                            b * TB:(b + 1) * TB])
        x_b.append(xb)

    qp1 = qpp.tile([HM, T], F32R, tag="qp", name=f"qp1{sfx}")
    feat_phase(x_b, wqe1, wke1, qp1, arin1, f"a1{sfx}")
    allreduce(arin1, arout1)

    # overlap with AR1: qp2 from enc_output (streamed)
    qp2 = qpp.tile([HM, T], F32R, tag="qp", name=f"qp2{sfx}")
    with ExitStack() as ph:
        ep = ph.enter_context(tc.tile_pool(name=f"encp{sfx}", bufs=2))
        pqs = ph.enter_context(tc.tile_pool(name=f"pq2{sfx}", bufs=2, space="PSUM"))
        for b in range(B):
            eb = ep.tile([128, DC * TB], F32R, tag="enc", name=f"enc{sfx}")
            for kc in range(DC):
                nc.sync.dma_start(eb[:, kc * TB:(kc + 1) * TB],
                                  tensors['encT'][kc * 128:(kc + 1) * 128,
                                                  b * TB:(b + 1) * TB])
            pq = pqs.tile([128, TB], F32, tag="pq2", name=f"pq2{sfx}")
            for kc in range(DC):
                _mm(nc, pq[:], wqe2[:, kc * HM:(kc + 1) * HM],
                    eb[:, kc * TB:(kc + 1) * TB], kc == 0, kc == DC - 1)
            nc.vector.tensor_scalar(qp2[:, b * TB:(b + 1) * TB], pq[:],
                                    0.0, STAB, AluOpType.max, AluOpType.add)
    wqk1.close()

    out1_b = attn_out_phase(tensors['wv1'], tensors['wo1'], arout1, qp1, x_b,
                            resid, "resid", F32R, 0, f"a1{sfx}")

    wqk2 = ExitStack()
    wqkp2 = wqk2.enter_context(tc.tile_pool(name=f"wk2p{sfx}", bufs=1))
    wke2 = load_wide(wqkp2, tensors['wke2'], HM, f"wke2{sfx}")
    feat_phase(out1_b, None, wke2, None, arin2, f"a2{sfx}")
    allreduce(arin2, arout2)
    wqk2.close()

    out2_b = attn_out_phase(tensors['wv2'], tensors['wo2'], arout2, qp2,
                            out1_b, o2pool, "o2", BF16, 1, f"a2{sfx}")
    mid.close()

    # ================= FFN (h = elu(out2 @ W1 + b1), spilled bf16) ==========
    SL = 512
    NSL = T // SL
    with ExitStack() as ph:
        wp = ph.enter_context(tc.tile_pool(name=f"w1p{sfx}", bufs=1))
        hstg = ph.enter_context(tc.tile_pool(name=f"hstg{sfx}", bufs=3))
        ep_ = ph.enter_context(tc.tile_pool(name=f"ep{sfx}", bufs=3))
        hps = ph.enter_context(tc.tile_pool(name=f"hps{sfx}", bufs=4, space="PSUM"))
        w1_sb = load_wide(wp, tensors['w1'], DFF, f"w1{sfx}", BF16)
        for s in range(NSL):
            b = s // (TB // SL)
            off = (s % (TB // SL)) * SL
            o2b = out2_b[b]
            for dffc in range(DFC):
                hps_t = hps.tile([128, SL], F32, tag="h", name=f"hps{sfx}")
                for kc in range(DC):
                    _mmb(nc, hps_t[:],
                         w1_sb[:, kc * DFF + dffc * 128:kc * DFF + dffc * 128 + 128],
                         o2b[:, kc * TB + off:kc * TB + off + SL],
                         kc == 0, kc == DC - 1)
                # ELU(u + b1) = min(exp(u+b1) - 1, max(u+b1, 0))
                e_ = ep_.tile([128, SL], F32, tag="e", name=f"e{sfx}")
                nc.scalar.activation(e_[:], hps_t[:], AF.Exp,
                                     bias=b1c[:, dffc:dffc + 1])
                t_ = ep_.tile([128, SL], F32, tag="t", name=f"t{sfx}")
                nc.scalar.activation(t_[:], hps_t[:], AF.Relu,
                                     bias=b1c[:, dffc:dffc + 1])
                h_ = hstg.tile([128, SL], BF16, tag="hsb", name=f"h{sfx}")
                nc.vector.scalar_tensor_tensor(h_[:], in0=e_[:], scalar=1.0,
                                               in1=t_[:], op0=AluOpType.subtract,
                                               op1=AluOpType.min)
                nc.sync.dma_start(
                    h_spill[dffc * 128:(dffc + 1) * 128,
                            s * SL:(s + 1) * SL], h_[:])

    # ============ r3 = h @ W2 + b2 + out2 ; token-major LN3 -> out ==========
    with ExitStack() as ph:
        wp = ph.enter_context(tc.tile_pool(name=f"w2p{sfx}", bufs=1))
        hin = ph.enter_context(tc.tile_pool(name=f"hin{sfx}", bufs=2))
        r3p = ph.enter_context(tc.tile_pool(name=f"r3p{sfx}", bufs=2))
        o3p = ph.enter_context(tc.tile_pool(name=f"o3p{sfx}", bufs=2))
        sqp = ph.enter_context(tc.tile_pool(name=f"sq3{sfx}", bufs=1))
        stp = ph.enter_context(tc.tile_pool(name=f"st3{sfx}", bufs=8))
        rps = ph.enter_context(tc.tile_pool(name=f"rps{sfx}", bufs=2, space="PSUM"))
        ops = ph.enter_context(tc.tile_pool(name=f"ops{sfx}", bufs=2, space="PSUM"))
        w2_sb = load_wide(wp, tensors['w2'], D, f"w2{sfx}", BF16)

        for s in range(NSL):
            b = s // (TB // SL)
            off = (s % (TB // SL)) * SL
            o2b = out2_b[b]
            h_sb = hin.tile([128, DFC * SL], BF16, tag="hin", name=f"hin{sfx}")
            for dffc in range(DFC):
                nc.sync.dma_start(h_sb[:, dffc * SL:(dffc + 1) * SL],
                                  h_spill[dffc * 128:(dffc + 1) * 128,
                                          s * SL:(s + 1) * SL])
            for t3 in range(SL // 128):
                toff = off + t3 * 128
                tok0 = b * TB + toff
                rt = [rps.tile([128, 512], F32, tag="r3", name=f"r3{sfx}{half}")
                      for half in range(2)]
                for half in range(2):
                    for dffc in range(DFC):
                        _mmb(nc, rt[half][:],
                             h_sb[:, dffc * SL + t3 * 128:dffc * SL + t3 * 128 + 128],
                             w2_sb[:, dffc * D + half * 512:dffc * D + half * 512 + 512],
                             dffc == 0, False, True)
                    _mm(nc, rt[half][:], ones_row[:],
                        b2r[:, half * 512:half * 512 + 512], False, True, True)
                # transpose out2 block (bf16) for the token-major residual
                o2t = ops.tile([128, D], BF16, tag="o2t", name=f"o2t{sfx}")
                for kc in range(DC):
                    nc.tensor.matmul(o2t[:, kc * 128:(kc + 1) * 128],
                                     o2b[:, kc * TB + toff:kc * TB + toff + 128],
                                     identb[:], start=True, stop=True,
                                     is_transpose=True, skip_group_check=True)
                o2ts = o3p.tile([128, D], BF16, tag="o2ts", name=f"o2ts{sfx}")
                nc.any.tensor_copy(o2ts[:], o2t[:])
                r3 = r3p.tile([128, D], F32, tag="r3s", name=f"r3s{sfx}")
                for half in range(2):
                    nc.vector.tensor_tensor(r3[:, half * 512:(half + 1) * 512],
                                            rt[half][:],
                                            o2ts[:, half * 512:(half + 1) * 512],
                                            AluOpType.add)
                # token-major LN3: stats along the free (feature) dim
                Sc = stp.tile([128, 1], F32, tag="st3", name=f"Sc{sfx}")
                nc.vector.tensor_reduce(Sc[:], r3[:], AX.X, AluOpType.add)
                mneg = stp.tile([128, 1], F32, tag="st3", name=f"mneg3{sfx}")
                nc.vector.tensor_scalar(mneg[:], Sc[:], -1.0 / D, None,
                                        AluOpType.mult)
                sq = sqp.tile([128, D], F32R, tag="sq3", name=f"sq3{sfx}")
                nc.scalar.activation(sq[:], r3[:], AF.Square)
                SSc = stp.tile([128, 1], F32, tag="st3", name=f"SSc{sfx}")
                nc.vector.tensor_reduce(SSc[:], sq[:], AX.X, AluOpType.add)
                m2 = stp.tile([128, 1], F32, tag="st3", name=f"m23{sfx}")
                nc.vector.tensor_tensor(m2[:], mneg[:], mneg[:], AluOpType.mult)
                ve = stp.tile([128, 1], F32, tag="st3", name=f"ve3{sfx}")
                nc.vector.scalar_tensor_tensor(ve[:], in0=SSc[:], scalar=1.0 / D,
                                               in1=m2[:], op0=AluOpType.mult,
                                               op1=AluOpType.subtract)
                sqv = stp.tile([128, 1], F32, tag="st3", name=f"sqv3{sfx}")
                nc.scalar.activation(sqv[:], ve[:], AF.Sqrt, bias=eps_c[:])
                a_ = stp.tile([128, 1], F32, tag="st3", name=f"a3{sfx}")
                nc.vector.reciprocal(a_[:], sqv[:])
                n1 = stp.tile([128, 1], F32, tag="st3", name=f"n13{sfx}")
                nc.vector.tensor_tensor(n1[:], a_[:], a_[:], AluOpType.mult)
                n2 = stp.tile([128, 1], F32, tag="st3", name=f"n23{sfx}")
                nc.vector.scalar_tensor_tensor(n2[:], in0=ve[:], scalar=EPS_LN,
                                               in1=n1[:], op0=AluOpType.add,
                                               op1=AluOpType.mult)
                nc.vector.tensor_scalar(n2[:], n2[:], -0.5, 1.5,
                                        AluOpType.mult, AluOpType.add)
                nc.vector.tensor_tensor(a_[:], a_[:], n2[:], AluOpType.mult)
                # apply: out = ((r3 - m) * rstd) * g3 + be3
                o3 = o3p.tile([128, D], F32, tag="o3", name=f"o3{sfx}")
                nc.vector.tensor_scalar(o3[:], r3[:], mneg[:], a_[:],
                                        AluOpType.add, AluOpType.mult)
                nc.gpsimd.tensor_tensor(o3[:], o3[:], g3bc[:], AluOpType.mult)
                nc.gpsimd.tensor_tensor(o3[:], o3[:], be3bc[:], AluOpType.add)
                nc.sync.dma_start(out_d[tok0:tok0 + 128, :], o3[:])
    o2p.close()


def _host_prep(inputs):
    """Build per-core in_maps from full inputs."""
    f32 = np.float32
    bf16 = ml_dtypes.bfloat16
    x = np.asarray(inputs['x'], f32)
    enc = np.asarray(inputs['enc_output'], f32)

    def fold(Wq, P):
        # [D, H, DH] x [M, DH] -> [D, H*M]
        w = np.einsum('dhk,mk->dhm', np.asarray(Wq, f32), np.asarray(P, f32))
        return np.ascontiguousarray(w.reshape(D, HM) / np.sqrt(M))

    e16T = np.zeros((HM, H), f32)
    e16 = np.zeros((H, HM), f32)
    for h in range(H):
        e16T[h * M:(h + 1) * M, h] = 1.0
        e16[h, h * M:(h + 1) * M] = 1.0

    khead = np.arange(D) // DH         # head of each v/k feature row
    fhead = np.arange(HM) // M         # head of each random feature
    kvm4 = (khead[:, None] == fhead[None, :]).astype(bf16)

    gbe = np.zeros((128, 4 * DC), f32)
    for i, nm in enumerate(['g1', 'be1', 'g2', 'be2']):
        gbe[:, i * DC:(i + 1) * DC] = np.asarray(inputs[nm], f32).reshape(DC, 128).T

    shared = {
        'wqe1': fold(inputs['Wq1'], inputs['P1']),
        'wke1': fold(inputs['Wk1'], inputs['P1']),
        'wqe2': fold(inputs['Wq2'], inputs['P2']),
        'wke2': fold(inputs['Wk2'], inputs['P2']),
        'wv1': np.asarray(inputs['Wv1'], f32).reshape(D, D).astype(bf16),
        'wo1': np.asarray(inputs['Wo1'], f32).reshape(D, D).astype(bf16),
        'wv2': np.asarray(inputs['Wv2'], f32).reshape(D, D).astype(bf16),
        'wo2': np.asarray(inputs['Wo2'], f32).reshape(D, D).astype(bf16),
        'w1': np.asarray(inputs['W1'], f32).astype(bf16),
        'w2': np.asarray(inputs['W2'], f32).astype(bf16),
        'kvm4': kvm4,
        'e16T': e16T, 'e16': e16,
        'gbe': gbe,
        'b1c': np.ascontiguousarray(np.asarray(inputs['b1'], f32).reshape(DFC, 128).T),
        'b2r': np.asarray(inputs['b2'], f32).reshape(1, D),
        'g3r': np.asarray(inputs['g3'], f32).reshape(1, D),
        'be3r': np.asarray(inputs['be3'], f32).reshape(1, D),
        'ident': np.eye(128, dtype=f32),
        'identb': np.eye(128, dtype=bf16),
        'ones_col': np.ones((128, 8), f32),
        'ones_row': np.ones((1, 128), f32),
    }

    in_maps = []
    for i in range(NCORES):
        sl = slice(i * LSH, (i + 1) * LSH)
        m = dict(shared)
        m['xT'] = np.ascontiguousarray(
            x[:, sl, :].transpose(2, 0, 1).reshape(D, T))
        m['encT'] = np.ascontiguousarray(
            enc[:, sl, :].transpose(2, 0, 1).reshape(D, T))
        in_maps.append(m)
    return in_maps


def kernel(**inputs) -> np.ndarray:
    if 'nc' not in _cache:
        _cache['nc'] = build_program()
    nc = _cache['nc']
    in_maps = _host_prep(inputs)
    res = run_bass_kernel_spmd(nc, in_maps, core_ids=list(range(NCORES)))
    out = np.empty((B, L, D), np.float32)
    for i in range(NCORES):
        o = res.results[i]['out']  # [T, D] token-major
        out[:, i * LSH:(i + 1) * LSH, :] = o.reshape(B, LSH, D)
    return out


if __name__ == '__main__':
    print("building program...")
    build_program()
    print("OK")
